# revision 70
# baseline (speedup 1.0000x reference)
"""Trainium2 Bass kernel for nn_GCN_5403068858882 (GCN + 3x GENConv + pool head).

Self-contained: schedule builder + bass program builder + SPMD runner.

Design:
- 8 cores, core c owns graphs [32c, 32c+32) (contiguous nodes, batch sorted).
- Nodes packed into 32-slot bins (cap TA*128 "A" edges / TB*128 "B" edges,
  A = src graph < G/2 so dma_gather int16 indices fit a half table).
- Selection matrices (one-hot of edge->dst-slot) are precomputed on the host
  in fp8 and loaded once as resident SBUF inputs; the PE matmul pairs them
  with the bf16 gathered rows (mixed-dtype matmul), so no on-device
  is_equal selection builds exist.
- Per GEN layer: node-space LN -> v=relu(u)+eps (before the in-place prelu,
  valid for positive slopes) -> bf16 table rows [e, v*e] -> AllGather
  (double-buffered across layers) -> per 128-edge tile: dma_gather rows
  (256B payloads; 768-idx calls, the largest the runtime-fixed SWDGE
  descriptor ring accepts) + PE matmul with the resident selection matrix
  accumulating numerator/denominator in PSUM -> agg=w/s+u -> MLP (bn
  folded, bf16) -> residual ledger (bf16).
- The node phase of layer i+1 is emitted per-superchunk inside layer i's
  edge phase (post_chunk) so it overlaps the gather stream.
- GCN conv: same machinery with narrow 128B fetches of bf16 h0*(deg^-1/2)
  rows (dinv folded into x on the host); self loop via own-row add.
- Pooling: bf16 SBUF-source dma_gather (transpose) straight from the bf16
  ledger into a per-graph padded channel-major grid, with per-slab reduces
  trailing the gather stream -> tiny AllGather -> MLP head.
"""

import numpy as np
import ml_dtypes

import concourse.ap_utils as ap_utils
import concourse.bass as bass
import concourse.bacc as bacc
import concourse.mybir as mybir
import concourse.tile as tile
from concourse.bass import MemorySpace
from concourse.bass_utils import run_bass_kernel_spmd
from concourse._compat import exact_div, get_trn_type, round_up_to_multiple

F32 = mybir.dt.float32
BF16 = mybir.dt.bfloat16
FP8 = mybir.dt.float8e4
I16 = mybir.dt.int16
AF = mybir.ActivationFunctionType
ALU = mybir.AluOpType
NPBF = ml_dtypes.bfloat16
NPF8 = ml_dtypes.float8_e4m3

H = 64
F_IN = 5
L = 3
EPS_BN = 1e-5
EPS_MSG = 1e-7
NCORES = 8
TA = 3
TB = 3
BINCAP = 32
CHUNK_BINS = 16          # bins per gather superchunk
MOCK_COLLECTIVES = False  # replace AllGathers with local DMA (TimelineSim)
NARROW_GATHER = True      # fetch 128B payloads from 256B-stride tables
GATHER_SPLIT = 8          # sub-calls per superchunk gather: 768-idx calls
                          # are the largest the runtime-fixed SWDGE
                          # descriptor ring accepts (larger calls deadlock
                          # the firmware's await_space)
DMA_SCRATCH = 16384       # SWDGE descriptor carveout bytes/partition


# ---------------------------------------------------------------- schedule
class Sched:
    pass


def build_schedule(edge_index, batch_idx, G):
    s = Sched()
    src = np.asarray(edge_index[0], np.int64)
    dst = np.asarray(edge_index[1], np.int64)
    batch = np.asarray(batch_idx, np.int64)
    n = batch.shape[0]
    s.G = G
    s.GPC = GPC = G // NCORES

    deg = np.bincount(dst, minlength=n).astype(np.float64) + 1.0
    s.dinv_node = (deg ** -0.5).astype(np.float32)

    a_edge = batch[src] < (G // 2)
    acnt = np.bincount(dst[a_edge], minlength=n)
    bcnt = np.bincount(dst[~a_edge], minlength=n)

    gstart = np.searchsorted(batch, np.arange(G))
    gend = np.searchsorted(batch, np.arange(G), side="right")
    s.cnt = cnt = gend - gstart

    CAP_A, CAP_B = TA * 128, TB * 128
    core_bins = []
    for c in range(NCORES):
        lo, hi = gstart[c * GPC], gend[(c + 1) * GPC - 1]
        bins, cur, ca, cb = [], [-1, -1], 0, 0
        for nd in range(lo, hi):
            if len(cur) >= BINCAP or ca + acnt[nd] > CAP_A or cb + bcnt[nd] > CAP_B:
                bins.append(cur)
                cur, ca, cb = [], 0, 0
            cur.append(nd)
            ca += acnt[nd]
            cb += bcnt[nd]
        bins.append(cur)
        core_bins.append(bins)

    NB = max(len(b) for b in core_bins)
    NB = -(-NB // CHUNK_BINS) * CHUNK_BINS
    s.NB = NB
    s.NSLOT = NSLOT = NB * BINCAP
    s.NBLK = NB // 4
    assert 4 * NSLOT <= 32768, NSLOT

    slot2node = np.full((NCORES, NSLOT), -1, np.int64)
    pos_of_node = np.full(n, -1, np.int64)
    for c in range(NCORES):
        for bi, bn in enumerate(core_bins[c]):
            for j, nd in enumerate(bn):
                if nd >= 0:
                    slot2node[c, bi * BINCAP + j] = nd
                    pos_of_node[nd] = c * NSLOT + bi * BINCAP + j
    assert (pos_of_node >= 0).all()
    s.slot2node, s.pos_of_node = slot2node, pos_of_node
    s.SPLIT = 4 * NSLOT

    dst_pos = pos_of_node[dst]
    dst_core = dst_pos // NSLOT
    dst_bin = (dst_pos % NSLOT) // BINCAP
    dst_slot = (dst_pos % NSLOT) % BINCAP
    src_pos = pos_of_node[src]

    NT_A, NT_B = NB * TA, NB * TB
    idxA = np.zeros((NCORES, NT_A * 128), np.int16)
    dstA = np.full((NCORES, NT_A * 128), -1.0, np.float32)
    idxB = np.zeros((NCORES, NT_B * 128), np.int16)
    dstB = np.full((NCORES, NT_B * 128), -1.0, np.float32)

    order = np.lexsort((src_pos, dst_bin, dst_core))
    eo_src, eo_core = src_pos[order], dst_core[order]
    eo_bin, eo_slot, eo_a = dst_bin[order], dst_slot[order], a_edge[order]

    for c in range(NCORES):
        msk_c = eo_core == c
        for idxarr, dstarr, T, off, grp in (
            (idxA, dstA, TA, 0, True),
            (idxB, dstB, TB, s.SPLIT, False),
        ):
            msk = msk_c & (eo_a == grp)
            bins_e, srcs, slots = eo_bin[msk], eo_src[msk] - off, eo_slot[msk]
            bs = np.searchsorted(bins_e, np.arange(NB))
            be = np.searchsorted(bins_e, np.arange(NB), side="right")
            for bi in range(NB):
                k = be[bi] - bs[bi]
                assert k <= T * 128
                base = bi * T * 128
                idxarr[c, base : base + k] = srcs[bs[bi] : be[bi]].astype(np.int16)
                dstarr[c, base : base + k] = slots[bs[bi] : be[bi]].astype(np.float32)

    s.idxA, s.idxB = idxA, idxB
    # host-built one-hot selection matrices, fp8
    # [K*128] dst codes -> [128, K, 32] one-hot (partition = edge in tile)
    iot = np.arange(BINCAP, dtype=np.float32)

    def onehot(dstarr, ntiles):
        codes = dstarr.reshape(ntiles, 128).T            # [128, ntiles]
        return (codes[:, :, None] == iot[None, None, :]).astype(NPF8)

    s.selA = np.stack([onehot(dstA[c], NT_A) for c in range(NCORES)])
    s.selB = np.stack([onehot(dstB[c], NT_B) for c in range(NCORES)])

    valid = slot2node >= 0
    s.valid = valid
    s.dinv_slot = np.where(
        valid, s.dinv_node[np.clip(slot2node, 0, None)], 0.0
    ).astype(np.float32)
    s.mask_slot = valid.astype(np.float32)

    maxcnt = int(cnt.max())
    SG = max(64, -(-maxcnt // 64) * 64)   # %64 so 2-graph pool gathers are %128
    s.SG = SG
    gidx_mean = np.zeros((NCORES, GPC * SG), np.int16)
    gidx_max = np.zeros((NCORES, GPC * SG), np.int16)
    for c in range(NCORES):
        for gl in range(GPC):
            g = c * GPC + gl
            slots = (pos_of_node[np.arange(gstart[g], gend[g])] % NSLOT).astype(
                np.int16
            )
            base = gl * SG
            gidx_mean[c, base : base + len(slots)] = slots
            gidx_max[c, base : base + len(slots)] = slots
            gidx_mean[c, base + len(slots) : base + SG] = 1
            gidx_max[c, base + len(slots) : base + SG] = 0
    s.gidx_mean, s.gidx_max = gidx_mean, gidx_max
    s.inv_cnt = (1.0 / np.maximum(cnt, 1)).astype(np.float32)
    s.maxmask = (cnt > 0).astype(np.float32)
    return s


def fold_weights(w):
    f = {}
    w32 = {k: np.asarray(v, np.float32) if np.asarray(v).dtype != np.int64 else v
           for k, v in w.items()}
    sbn1 = w32["bn1_g"] / np.sqrt(1.0 + EPS_BN)
    f["Wc"] = (w32["conv1_W"] * sbn1[None, :]).astype(np.float32)
    f["btot_conv"] = (w32["conv1_b"] * sbn1 + w32["bn1_b"]).astype(np.float32)
    f["ln_g"], f["ln_b"] = w32["ln_g"], w32["ln_b"]
    f["prelu_a"], f["gen_t"] = w32["prelu_a"], w32["gen_t"]
    # v-table fast path: relu(prelu(u)) == relu(u) elementwise iff slope > 0,
    # so the message v can be computed before the in-place prelu.
    assert (w32["prelu_a"] > 0).all(), "kernel assumes positive prelu slopes"
    f["g_unit"] = [bool((w32["ln_g"][i] == 1.0).all()) for i in range(L)]
    f["b_zero"] = [bool((w32["ln_b"][i] == 0.0).all()) for i in range(L)]
    f["a_scalar"] = [
        float(w32["prelu_a"][i][0])
        if (w32["prelu_a"][i] == w32["prelu_a"][i][0]).all() else None
        for i in range(L)
    ]
    f["W1"], f["b1tot"], f["W2"], f["b2"] = [], [], [], []
    for i in range(L):
        smlp = w32["mlp_bn_g"][i] / np.sqrt(1.0 + EPS_BN)
        f["W1"].append((w32["mlp_W1"][i] * smlp[None, :]).astype(NPBF))
        f["b1tot"].append(
            (w32["mlp_b1"][i] * smlp + w32["mlp_bn_b"][i]).astype(np.float32)
        )
        f["W2"].append(w32["mlp_W2"][i].astype(NPBF))
        f["b2"].append(w32["mlp_b2"][i])
    for k in ("lin1_W", "lin1_b", "lin2_W", "lin2_b", "out_W", "out_b"):
        f[k] = w32[k]
    return f


def _wrap16(arr):
    """[K*16] -> [128, K] gather-idx layout (i at [i%16, i//16], tiled x8)."""
    a = np.asarray(arr, np.int16).reshape(-1, 16).T  # [16, K]
    return np.tile(a, (8, 1)).copy()


def build_inmaps(s, x):
    NSLOT, NBLK = s.NSLOT, s.NBLK
    maps = []
    for c in range(NCORES):
        # dinv folded in on the host: h0n = (x * dinv) @ Wc
        xpad = np.zeros((NSLOT, F_IN), np.float32)
        v = s.valid[c]
        xpad[v] = (np.asarray(x, np.float32)[s.slot2node[c][v]]
                   * s.dinv_slot[c][v][:, None])
        m = {
            "xT": np.ascontiguousarray(xpad.T),
            "idxA": _wrap16(s.idxA[c]),
            "idxB": _wrap16(s.idxB[c]),
            "selA": np.ascontiguousarray(s.selA[c]),
            "selB": np.ascontiguousarray(s.selB[c]),
            "dinv": np.ascontiguousarray(
                s.dinv_slot[c].reshape(NBLK, 128).T
            ),
            "mask": np.ascontiguousarray(
                s.mask_slot[c].reshape(NBLK, 128).T
            ),
            "gidxm": _wrap16(s.gidx_mean[c]),
            "gidxx": _wrap16(s.gidx_max[c]),
            "pminv": np.tile(s.inv_cnt[c * s.GPC : (c + 1) * s.GPC], (128, 1)).astype(np.float32),
            "pmax": np.tile(s.maxmask[c * s.GPC : (c + 1) * s.GPC], (128, 1)).astype(np.float32),
        }
        maps.append(m)
    return maps


def _dma_gather_narrow(gp, out_ap, in_ap, idxs_ap, num_idxs, num_idxs_reg,
                       elem_size, elem_step, queue_num):
    """bass.GpSimd.dma_gather fork: non-transpose DRAM-source gather whose
    fetched payload (elem_size) is narrower than the 256B-granular row stride
    (elem_step). The stock API asserts elem_size_bytes % 256 == 0, but that is
    a transpose-mode firmware restriction; the non-transpose descriptor
    generator emits one elem_size_bytes descriptor per index with the source
    address advancing by stride_bytes_256*256 per index, so a narrow payload
    on a padded-stride table is well-formed."""
    gp._assert_queue_num(queue_num)
    assert idxs_ap.dtype == mybir.dt.int16
    assert in_ap.space == MemorySpace.DRAM
    assert idxs_ap.space == MemorySpace.SBUF and out_ap.space == MemorySpace.SBUF
    assert in_ap.dtype == out_ap.dtype
    assert ap_utils.ap_is_contiguous(out_ap.ap[1:])
    assert ap_utils.ap_is_contiguous(idxs_ap.ap[1:])
    assert in_ap.ap[-1][1] == out_ap.ap[-1][1] == elem_size
    assert out_ap.ap[0][1] * out_ap.ap[1][1] == round_up_to_multiple(num_idxs, 128)
    assert in_ap.ap[0][0] == elem_step
    stride_bytes_256 = exact_div(elem_step * mybir.dt.size(in_ap.dtype), 256)
    assert stride_bytes_256 < 256
    return gp.add_instruction(
        mybir.InstDMAGatherAnt(
            name=gp.bass.get_next_instruction_name(),
            ins=[
                *gp.lower_ap_dma(in_ap, for_custom_bir_dma=True),
                gp.lower_ap(idxs_ap),
                gp.lower_val_access(gp.to_reg(num_idxs_reg)),
            ],
            outs=[gp.lower_ap(out_ap)],
            transpose=False,
            num_idxs=num_idxs,
            elem_size=elem_size,
            stride_bytes_256=stride_bytes_256,
            gen_mode=0,
            single_packet=True,
            queue_num=queue_num,
            sbuf_tokens_per_rank=0,
            sbuf_free_dim_per_rank=0,
            sbuf_free_dim_pad_per_rank=0,
            sbuf_byte_offset=0,
        )
    )


# ---------------------------------------------------------------- bass build
def build_nc(s, f):
    NB, NSLOT, NBLK, SG, GPC = s.NB, s.NSLOT, s.NBLK, s.SG, s.GPC
    NSC = NB // CHUNK_BINS
    NT_CH_A = CHUNK_BINS * TA            # tiles per A-chunk (48)
    NT_CH_B = CHUNK_BINS * TB
    NIDX_A = NT_CH_A * 128
    NIDX_B = NT_CH_B * 128
    NTA, NTB = NB * TA, NB * TB

    nc = bacc.Bacc(get_trn_type() or "TRN2", num_devices=NCORES,
                   num_swdge_queues=2, dynamic_dma_scratch_size=DMA_SCRATCH)

    # ---- I/O ----
    xT_d = nc.dram_tensor("xT", [F_IN, NSLOT], F32, kind="ExternalInput")
    idxA_d = nc.dram_tensor("idxA", [128, NTA * 8], I16, kind="ExternalInput")
    idxB_d = nc.dram_tensor("idxB", [128, NTB * 8], I16, kind="ExternalInput")
    selA_d = nc.dram_tensor("selA", [128, NTA, 32], FP8, kind="ExternalInput")
    selB_d = nc.dram_tensor("selB", [128, NTB, 32], FP8, kind="ExternalInput")
    dinv_d = nc.dram_tensor("dinv", [128, NBLK], F32, kind="ExternalInput")
    mask_d = nc.dram_tensor("mask", [128, NBLK], F32, kind="ExternalInput")
    gidxm_d = nc.dram_tensor("gidxm", [128, GPC * SG // 16], I16, kind="ExternalInput")
    gidxx_d = nc.dram_tensor("gidxx", [128, GPC * SG // 16], I16, kind="ExternalInput")
    pminv_d = nc.dram_tensor("pminv", [128, GPC], F32, kind="ExternalInput")
    pmax_d = nc.dram_tensor("pmax", [128, GPC], F32, kind="ExternalInput")
    out_d = nc.dram_tensor("out", [s.G, 1], F32, kind="ExternalOutput")

    # ---- shared consts ----
    it = nc.inline_tensor
    Wc_d = it(f["Wc"], "Wc")                                     # [5,64]
    btotb_d = it(np.tile(f["btot_conv"], (128, 1)), "btotb")     # [128,64]
    W1_d = [it(f["W1"][i], f"W1_{i}") for i in range(L)]         # [64,128] bf16
    W2_d = [it(f["W2"][i], f"W2_{i}") for i in range(L)]         # [128,64] bf16
    b1_d = [it(f["b1tot"][i][:, None], f"b1_{i}") for i in range(L)]   # [128,1]
    b2b_d = [it(np.tile(f["b2"][i], (128, 1)), f"b2b_{i}") for i in range(L)]
    gbb_d = [it(np.tile(f["ln_g"][i], (128, 1)), f"gbb_{i}") for i in range(L)]
    bbb_d = [it(np.tile(f["ln_b"][i], (128, 1)), f"bbb_{i}") for i in range(L)]
    abb_d = [it(np.tile(f["prelu_a"][i], (128, 1)), f"abb_{i}") for i in range(L)]
    l1W_d = [it(np.ascontiguousarray(f["lin1_W"][k * 128 : (k + 1) * 128]), f"l1W_{k}") for k in range(4)]
    l1b_d = it(f["lin1_b"][:, None], "l1b")                      # [128,1]
    l2W_d = it(f["lin2_W"], "l2W")                               # [128,64]
    l2b_d = it(f["lin2_b"][:, None], "l2b")                      # [64,1]
    oW_d = it(f["out_W"], "oW")                                  # [64,1]
    ident_d = it(np.eye(128, dtype=np.float32), "ident")

    # ---- internal DRAM ----
    # table rows at 256B stride (gather row-stride granularity): [h0n|-]
    # during conv, [e|ve] during GEN. Double-buffered per layer so layer
    # i+1's AllGather overlaps the tail of layer i's gather stream.
    ag_in = [nc.dram_tensor(f"ag_in{j}", [NSLOT, 128], BF16) for j in range(2)]
    ag_out = [nc.dram_tensor(f"ag_out{j}", [NCORES * NSLOT, 128], BF16,
                             addr_space="Shared") for j in range(2)]
    pool_in = nc.dram_tensor("pool_in", [4, 128, GPC], F32)
    pool_out = nc.dram_tensor("pool_out", [NCORES, 4, 128, GPC], F32, addr_space="Shared")

    RG = [list(range(NCORES))]

    def allgather(cin, cout):
        if MOCK_COLLECTIVES:
            nc.sync.dma_start(out=cout[0 : cin.shape[0]], in_=cin[:])
        else:
            nc.gpsimd.collective_compute(
                "AllGather", ALU.bypass, replica_groups=RG,
                ins=[cin[:]], outs=[cout[:]],
            )

    from contextlib import ExitStack

    with tile.TileContext(nc) as tc:
        with tc.tile_pool(name="persist", bufs=1) as pp:
            # message-passing-lifetime pool: closed before pooling so the
            # pool/head phase can reuse the space
            eph_ctx = ExitStack()
            eh = eph_ctx.enter_context(tc.tile_pool(name="eph", bufs=1))
            idxA_sb = eh.tile([128, NTA * 8], I16)
            nc.sync.dma_start(out=idxA_sb[:], in_=idxA_d[:, :])
            idxB_sb = eh.tile([128, NTB * 8], I16)
            nc.sync.dma_start(out=idxB_sb[:], in_=idxB_d[:, :])
            # sel tiles are declared here but loaded after the conv node
            # phase: they are first read by the conv edge matmuls, so the
            # ~56KB load must not delay the h0/x DMAs and first gathers
            selA = eh.tile([128, NTA, 32], FP8)
            selB = eh.tile([128, NTB, 32], FP8)
            dinv = pp.tile([128, NBLK], F32)
            nc.sync.dma_start(out=dinv[:], in_=dinv_d[:, :])
            mask = pp.tile([128, NBLK], F32)
            nc.sync.dma_start(out=mask[:], in_=mask_d[:, :])

            # consts
            _ldn = [0]

            def ld(dram, shape, dtype=F32):
                _ldn[0] += 1
                nm = f"c{_ldn[0]}_{dram.name}"
                t = pp.tile(shape, dtype, name=nm, tag=nm)
                nc.sync.dma_start(out=t[:], in_=dram[tuple(slice(None) for _ in shape)])
                return t

            Wc = ld(Wc_d, [F_IN, H])
            btotb = ld(btotb_d, [128, H])
            W1 = [ld(W1_d[i], [H, 2 * H], BF16) for i in range(L)]
            W2 = [ld(W2_d[i], [2 * H, H], BF16) for i in range(L)]
            b1 = [ld(b1_d[i], [128, 1]) for i in range(L)]
            b2b = [ld(b2b_d[i], [128, H]) for i in range(L)]
            gbb = [ld(gbb_d[i], [128, H]) for i in range(L)]
            bbb = [ld(bbb_d[i], [128, H]) for i in range(L)]
            abb = [ld(abb_d[i], [128, H]) for i in range(L)]
            l1W = [ld(l1W_d[k], [128, 128]) for k in range(4)]
            l1b = ld(l1b_d, [128, 1])
            l2W = ld(l2W_d, [128, H])
            l2b = ld(l2b_d, [H, 1])
            oW = ld(oW_d, [H, 1])
            ident = ld(ident_d, [128, 128])
            epsb = pp.tile([128, 1], F32)
            nc.vector.memset(epsb[:], EPS_BN)
            e30b = pp.tile([128, 1], F32)
            nc.vector.memset(e30b[:], 1e-30)

            # persistent state
            ledger = pp.tile([128, NBLK, (L + 1) * H], BF16)
            usc = eh.tile([128, NBLK, H], F32)       # h0n during conv, u in GEN
            ab = eh.tile([128, NBLK, 2 * H], BF16)   # table rows: [h0n|-]/[e|ve]

            assert NIDX_A == NIDX_B
            nidx_reg = nc.gpsimd.to_reg(NIDX_A // GATHER_SPLIT)

            def node_chunk(i, sc, ep):
                """LN -> v-table (+ per-chunk ag_in store) -> in-place prelu
                for GEN layer i over superchunk sc's 4 blocks. Emitted inside
                the previous edge phase so it overlaps the gather stream."""
                B0 = 4 * sc
                mv = ep.tile([128, 4, 2], F32, tag="nmv", bufs=2)
                for k in range(4):
                    blk = B0 + k
                    st = ep.tile([128, 6], F32, tag="nst", bufs=3)
                    nc.vector.bn_stats(out=st[:], in_=ledger[:, blk, i * H : (i + 1) * H])
                    nc.vector.bn_aggr(out=mv[:, k, :], in_=st[:])
                rstd = ep.tile([128, 4], F32, tag="nrstd", bufs=2)
                nc.scalar.activation(
                    out=rstd[:], in_=mv[:, :, 1], func=AF.Sqrt,
                    bias=epsb[:], scale=1.0,
                )
                nc.vector.reciprocal(out=rstd[:], in_=rstd[:])
                nmr = ep.tile([128, 4], F32, tag="nnmr", bufs=2)
                nc.vector.tensor_tensor(
                    out=nmr[:], in0=mv[:, :, 0], in1=rstd[:], op=ALU.mult
                )
                nc.vector.tensor_scalar(
                    out=nmr[:], in0=nmr[:], scalar1=-1.0, scalar2=None, op0=ALU.mult
                )
                for k in range(4):
                    blk = B0 + k
                    nc.vector.tensor_scalar(
                        out=usc[:, blk, :],
                        in0=ledger[:, blk, i * H : (i + 1) * H],
                        scalar1=rstd[:, k : k + 1],
                        scalar2=nmr[:, k : k + 1],
                        op0=ALU.mult, op1=ALU.add,
                    )
                u = usc[:, B0 : B0 + 4, :]
                if not f["g_unit"][i]:
                    gbig = gbb[i][:].unsqueeze(1).broadcast_to([128, 4, H])
                    nc.vector.tensor_tensor(out=u, in0=u, in1=gbig, op=ALU.mult)
                if not f["b_zero"][i]:
                    bbig = bbb[i][:].unsqueeze(1).broadcast_to([128, 4, H])
                    nc.vector.tensor_tensor(out=u, in0=u, in1=bbig, op=ALU.add)
                # v before the in-place prelu (valid for slopes > 0), then the
                # table rows [e, v*e] in bf16
                vt = ep.tile([128, 4, H], F32, tag="nr", bufs=2)
                nc.vector.tensor_scalar(
                    out=vt[:], in0=u, scalar1=0.0, scalar2=EPS_MSG,
                    op0=ALU.max, op1=ALU.add,
                )
                et = ep.tile([128, 4, H], F32, tag="net", bufs=2)
                nc.scalar.activation(
                    out=et[:], in_=vt[:], func=AF.Exp, scale=float(f["gen_t"][i]),
                )
                nc.vector.tensor_copy(out=ab[:, B0 : B0 + 4, 0:H], in_=et[:])
                nc.vector.tensor_tensor(
                    out=ab[:, B0 : B0 + 4, H : 2 * H], in0=vt[:], in1=et[:],
                    op=ALU.mult,
                )
                nc.sync.dma_start(
                    out=ag_in[(i + 1) % 2].ap()[
                        B0 * 128 : (B0 + 4) * 128, :
                    ].rearrange("(b p) c -> p b c", p=128),
                    in_=ab[:, B0 : B0 + 4, :],
                )
                # prelu in place for the root add; vt = relu(u) + 1e-7 stands
                # in for relu(u) (shift far below tolerance)
                if f["a_scalar"][i] is not None:
                    nc.vector.tensor_scalar(
                        out=u, in0=u, scalar1=0.0, scalar2=f["a_scalar"][i],
                        op0=ALU.min, op1=ALU.mult,
                    )
                else:
                    abig = abb[i][:].unsqueeze(1).broadcast_to([128, 4, H])
                    nc.vector.tensor_tensor(out=u, in0=u, in1=vt[:], op=ALU.subtract)
                    nc.vector.tensor_tensor(out=u, in0=u, in1=abig, op=ALU.mult)
                nc.vector.tensor_tensor(out=u, in0=u, in1=vt[:], op=ALU.add)

            def edge_phase(tag, nch, drain_fn, tbl, post_chunk=None):
                """Shared edge machinery over the bf16 table rows.

                Conv (nch=H): PSUM[slot, 0:H] = sum_e h0n[src] (narrow 128B
                fetch). GEN (nch=2H): PSUM[slot, :] = sum_e [e|ve][src].
                drain_fn(blk, psum, ep, mpp)."""
                fetch = nch
                with (
                    tc.tile_pool(name=f"ep_{tag}", bufs=1) as ep,
                    tc.tile_pool(name=f"epp_{tag}", bufs=3, space="PSUM") as epp,
                    tc.tile_pool(name=f"mpp_{tag}", bufs=1, space="PSUM") as mpp,
                ):
                    for sc in range(NSC):
                        ia = idxA_sb[:, sc * (NIDX_A // 16) : (sc + 1) * (NIDX_A // 16)]
                        ib = idxB_sb[:, sc * (NIDX_B // 16) : (sc + 1) * (NIDX_B // 16)]
                        ga = ep.tile([128, NT_CH_A, fetch], BF16, tag="ga", bufs=2)
                        gb = ep.tile([128, NT_CH_B, fetch], BF16, tag="gb", bufs=2)
                        GS = GATHER_SPLIT
                        tpc = NT_CH_A // GS      # tiles per sub-call
                        nn = tpc * 128
                        for k in range(GS):
                            _dma_gather_narrow(
                                nc.gpsimd, ga[:, k * tpc : (k + 1) * tpc, :],
                                tbl[0 : s.SPLIT, 0:fetch],
                                ia[:, k * (nn // 16) : (k + 1) * (nn // 16)],
                                nn, nidx_reg, fetch, 128, queue_num=0,
                            )
                            _dma_gather_narrow(
                                nc.gpsimd, gb[:, k * tpc : (k + 1) * tpc, :],
                                tbl[s.SPLIT : 2 * s.SPLIT, 0:fetch],
                                ib[:, k * (nn // 16) : (k + 1) * (nn // 16)],
                                nn, nidx_reg, fetch, 128, queue_num=1,
                            )
                        for bl in range(4):
                            blk = sc * 4 + bl
                            ps = epp.tile([128, nch], F32, tag="eps", space="PSUM")
                            for j in range(4):
                                lbin = bl * 4 + j       # bin within superchunk
                                for t in range(TA):
                                    gt = lbin * TA + t
                                    nc.tensor.matmul(
                                        out=ps[32 * j : 32 * j + 32, :],
                                        lhsT=selA[:, (sc * CHUNK_BINS + lbin) * TA + t, :],
                                        rhs=ga[:, gt, :],
                                        start=(t == 0),
                                        stop=False,
                                        tile_position=(0, 32 * j),
                                    )
                                for t in range(TB):
                                    gt = lbin * TB + t
                                    nc.tensor.matmul(
                                        out=ps[32 * j : 32 * j + 32, :],
                                        lhsT=selB[:, (sc * CHUNK_BINS + lbin) * TB + t, :],
                                        rhs=gb[:, gt, :],
                                        start=False,
                                        stop=(t == TB - 1),
                                        tile_position=(0, 32 * j),
                                    )
                            drain_fn(blk, ps, ep, mpp)
                        if post_chunk is not None:
                            post_chunk(sc, ep)

            # ================= conv =================
            with (
                tc.tile_pool(name="cvp", bufs=2, space="PSUM") as cvp,
                tc.tile_pool(name="cvs", bufs=1) as cvs,
            ):
                # one DMA for all of x; the cvs pool closes before the edge
                # pools open, so the tile doesn't stack with gather buffers
                xt_all = cvs.tile([F_IN, NSLOT], F32, tag="xt_all")
                nc.sync.dma_start(out=xt_all[:], in_=xT_d[:, :])
                for q in range(NBLK // 4):
                    h0ps = cvp.tile([128, 4, H], F32, space="PSUM")
                    for k in range(4):
                        blk = q * 4 + k
                        nc.tensor.matmul(
                            out=h0ps[:, k, :],
                            lhsT=xt_all[:, blk * 128 : (blk + 1) * 128],
                            rhs=Wc[:],
                            start=True, stop=True,
                        )
                    nc.vector.tensor_copy(
                        out=ab[:, q * 4 : q * 4 + 4, 0:H], in_=h0ps[:]
                    )
            nc.sync.dma_start(
                out=ag_in[0].ap()[:, 0:H].rearrange("(b p) c -> p b c", p=128),
                in_=ab[:, :, 0:H],
            )
            allgather(ag_in[0], ag_out[0])
            nc.sync.dma_start(out=selA[:], in_=selA_d[:, :, :])
            nc.sync.dma_start(out=selB[:], in_=selB_d[:, :, :])

            def conv_drain(blk, ps, ep, mpp):
                t1 = ep.tile([128, H], F32, tag="cd", bufs=3)
                nc.vector.tensor_add(t1[:], ps[:], ab[:, blk, 0:H])
                nc.vector.tensor_scalar(
                    out=t1[:], in0=t1[:],
                    scalar1=dinv[:, blk : blk + 1], scalar2=None, op0=ALU.mult,
                )
                nc.vector.tensor_add(t1[:], t1[:], btotb[:])
                nc.vector.tensor_scalar(
                    out=ledger[:, blk, 0:H], in0=t1[:],
                    scalar1=0.0, scalar2=mask[:, blk : blk + 1],
                    op0=ALU.max, op1=ALU.mult,
                )

            edge_phase("cv", H, conv_drain, tbl=ag_out[0],
                       post_chunk=lambda sc, ep: node_chunk(0, sc, ep))
            allgather(ag_in[1], ag_out[1])

            # ================= GEN layers =================
            # node phase for layer i is interleaved into the previous edge
            # phase (post_chunk); only the table AllGather sits between.
            for i in range(L):
                def gen_drain(blk, ps, ep, mpp, i=i):
                    sden = ep.tile([128, H], F32, tag="sden", bufs=3)
                    nc.vector.tensor_scalar(
                        out=sden[:], in0=ps[:, 0:H], scalar1=1e-30, scalar2=None,
                        op0=ALU.add,
                    )
                    nc.vector.reciprocal(out=sden[:], in_=sden[:])
                    agg = ep.tile([128, H], F32, tag="agg", bufs=3)
                    nc.vector.tensor_tensor(
                        out=agg[:], in0=ps[:, H : 2 * H], in1=sden[:], op=ALU.mult
                    )
                    nc.vector.tensor_add(agg[:], agg[:], usc[:, blk, :])
                    tps = mpp.tile([H, 128], F32, tag="tps", space="PSUM")
                    nc.tensor.transpose(out=tps[:], in_=agg[:], identity=ident[:])
                    aggT = ep.tile([H, 128], BF16, tag="aggT", bufs=3)
                    nc.vector.tensor_copy(out=aggT[:], in_=tps[:])
                    z1ps = mpp.tile([128, 128], F32, tag="z1", space="PSUM")
                    nc.tensor.matmul(
                        out=z1ps[:], lhsT=W1[i][:], rhs=aggT[:], start=True, stop=True
                    )
                    z1r = ep.tile([128, 128], BF16, tag="z1r", bufs=3)
                    nc.scalar.activation(
                        out=z1r[:], in_=z1ps[:], func=AF.Relu, bias=b1[i][:], scale=1.0
                    )
                    z2ps = mpp.tile([128, H], F32, tag="z2", space="PSUM")
                    nc.tensor.matmul(
                        out=z2ps[:], lhsT=z1r[:], rhs=W2[i][:], start=True, stop=True
                    )
                    t2 = ep.tile([128, H], F32, tag="t2", bufs=3)
                    nc.vector.tensor_add(t2[:], z2ps[:], b2b[i][:])
                    nc.vector.tensor_add(t2[:], t2[:], ledger[:, blk, i * H : (i + 1) * H])
                    nc.vector.tensor_scalar(
                        out=ledger[:, blk, (i + 1) * H : (i + 2) * H], in0=t2[:],
                        scalar1=mask[:, blk : blk + 1], scalar2=None, op0=ALU.mult,
                    )

                edge_phase(
                    f"g{i}", 2 * H, gen_drain, tbl=ag_out[(i + 1) % 2],
                    post_chunk=(
                        (lambda sc, ep, j=i + 1: node_chunk(j, sc, ep))
                        if i < L - 1 else None
                    ),
                )
                if i < L - 1:
                    allgather(ag_in[(i + 2) % 2], ag_out[(i + 2) % 2])

            # ================= pooling + head =================
            eph_ctx.close()
            CH = (L + 1) * H
            with (
                tc.tile_pool(name="pool", bufs=1) as qp,
                tc.tile_pool(name="poolps", bufs=2, space="PSUM") as qpp,
            ):
                gnidx_reg = nc.gpsimd.to_reg(2 * SG)
                nc.vector.memset(ledger[0:1, 0, 0:CH], -3.0e38)
                lbf = ledger
                pooled = qp.tile([128, 4, GPC], F32)
                PGS = 2 * SG                      # idxs per sub-call (<=768)
                gis, pscs, grids = [], [], []
                for which, gidx_d, pscale_d in (
                    (0, gidxm_d, pminv_d),
                    (1, gidxx_d, pmax_d),
                ):
                    gi = qp.tile([128, GPC * SG // 16], I16, tag=f"gi{which}")
                    nc.sync.dma_start(out=gi[:], in_=gidx_d[:, :])
                    psc = qp.tile([128, GPC], F32, tag=f"psc{which}")
                    nc.sync.dma_start(out=psc[:], in_=pscale_d[:, :])
                    grid = qp.tile([128, GPC // 2, 2, PGS], BF16, tag=f"grid{which}")
                    gis.append(gi)
                    pscs.append(psc)
                    grids.append(grid)
                # issue mean/max gathers interleaved on the two queues and
                # reduce each 2-graph slab as soon as its gather lands, so the
                # reduces trail the gather stream instead of following it
                for k in range(GPC // 2):
                    for which in (0, 1):
                        nc.gpsimd.dma_gather(
                            grids[which][:, k, :, :],
                            lbf[:].rearrange("p b c -> p (b c)"),
                            gis[which][:, k * (PGS // 16) : (k + 1) * (PGS // 16)],
                            PGS, gnidx_reg, CH,
                            transpose=True,
                            sbuf_tokens_per_rank=128,
                            sbuf_free_dim_per_rank=CH * 2,
                            queue_num=which,
                        )
                    for which in (0, 1):
                        red_op = (nc.vector.reduce_sum if which == 0
                                  else nc.vector.reduce_max)
                        for half in range(2):
                            red = qp.tile([128, 2], F32, tag="red", bufs=4)
                            red_op(
                                out=red[:].rearrange("p (k m) -> p k m", k=1),
                                in_=grids[which][:, k, half, :].rearrange(
                                    "p (m t) -> p () m t", t=SG
                                ),
                                axis=mybir.AxisListType.X,
                            )
                            nc.vector.tensor_tensor(
                                out=pooled[:, which * 2 + half, 2 * k : 2 * k + 2],
                                in0=red[:],
                                in1=pscs[which][:, 2 * k : 2 * k + 2],
                                op=ALU.mult,
                            )
                nc.sync.dma_start(
                    out=pool_in.ap().rearrange("k p g -> p k g"), in_=pooled[:]
                )
                if MOCK_COLLECTIVES:
                    nc.sync.dma_start(
                        out=pool_out[0, :, :, :], in_=pool_in[:, :, :]
                    )
                else:
                    nc.gpsimd.collective_compute(
                        "AllGather", ALU.bypass, replica_groups=RG,
                        ins=[pool_in[:, :, :]], outs=[pool_out[:, :, :, :]],
                    )
                # head
                hps = qpp.tile([128, s.G], F32, tag="hps", space="PSUM")
                pk = []
                for k in range(4):
                    t = qp.tile([128, NCORES, GPC], F32, tag=f"pk{k}")
                    nc.sync.dma_start(
                        out=t[:], in_=pool_out[:, k, :, :].rearrange("r p g -> p r g")
                    )
                    pk.append(t)
                for k in range(4):
                    nc.tensor.matmul(
                        out=hps[:], lhsT=l1W[k][:],
                        rhs=pk[k][:].rearrange("p r g -> p (r g)"),
                        start=(k == 0), stop=(k == 3),
                    )
                hz1 = qp.tile([128, s.G], F32)
                nc.scalar.activation(
                    out=hz1[:], in_=hps[:], func=AF.Relu, bias=l1b[:], scale=1.0
                )
                h2ps = qpp.tile([H, s.G], F32, tag="h2ps", space="PSUM")
                nc.tensor.matmul(out=h2ps[:], lhsT=l2W[:], rhs=hz1[:], start=True, stop=True)
                hz2 = qp.tile([H, s.G], F32)
                nc.scalar.activation(
                    out=hz2[:], in_=h2ps[:], func=AF.Relu, bias=l2b[:], scale=1.0
                )
                ops = qpp.tile([1, s.G], F32, tag="ops", space="PSUM")
                nc.tensor.matmul(out=ops[:], lhsT=oW[:], rhs=hz2[:], start=True, stop=True)
                osb = qp.tile([1, s.G], F32)
                nc.vector.tensor_scalar(
                    out=osb[:], in0=ops[:], scalar1=float(f["out_b"][0]),
                    scalar2=None, op0=ALU.add,
                )
                nc.sync.dma_start(out=out_d.ap().rearrange("g one -> one g"), in_=osb[:])

    nc.compile()
    return nc


# ---------------------------------------------------------------- entry
def kernel(**inputs) -> np.ndarray:
    x = np.asarray(inputs["x"], np.float32)
    ei = np.asarray(inputs["edge_index"], np.int64)
    bi = np.asarray(inputs["batch_idx"], np.int64)
    G = 256
    s = build_schedule(ei, bi, G)
    f = fold_weights(inputs)
    maps = build_inmaps(s, x)
    nc = build_nc(s, f)
    res = run_bass_kernel_spmd(nc, maps, core_ids=list(range(NCORES)))
    return np.asarray(res.results[0]["out"], np.float32)


# revision 74
# speedup vs baseline: 1.1345x; 1.1345x over previous
"""Trainium2 Bass kernel for nn_GCN_5403068858882 (GCN + 3x GENConv + pool head).

Self-contained: schedule builder + bass program builder + SPMD runner.

Design:
- 8 cores, core c owns graphs [32c, 32c+32) (contiguous nodes, batch sorted).
- Nodes packed into 32-slot bins (cap TA*128 "A" edges / TB*128 "B" edges,
  A = src graph < G/2 so dma_gather int16 indices fit a half table).
- Selection matrices (one-hot of edge->dst-slot) are precomputed on the host
  in fp8 and loaded once as resident SBUF inputs; the PE matmul pairs them
  with the bf16 gathered rows (mixed-dtype matmul), so no on-device
  is_equal selection builds exist.
- Per GEN layer: node-space LN -> v=relu(u)+eps (before the in-place prelu,
  valid for positive slopes) -> bf16 table rows [e, v*e] -> AllGather
  (double-buffered across layers) -> per 128-edge tile: dma_gather rows
  (256B payloads; 896-idx calls put 57 descriptors on each 64-entry
  per-DMA SWDGE ring, the firmware limit) + PE matmul with the resident selection matrix
  accumulating numerator/denominator in PSUM -> agg=w/s+u -> MLP (bn
  folded, bf16) -> residual ledger (bf16).
- The node phase of layer i+1 is emitted per-superchunk inside layer i's
  edge phase (post_chunk) so it overlaps the gather stream.
- GCN conv: same machinery with narrow 128B fetches of bf16 h0*(deg^-1/2)
  rows (dinv folded into x on the host); self loop via own-row add.
- Pooling: bf16 SBUF-source dma_gather (transpose) straight from the bf16
  ledger into a per-graph padded channel-major grid, with per-slab reduces
  trailing the gather stream -> tiny AllGather -> MLP head.
"""

import numpy as np
import ml_dtypes

import concourse.ap_utils as ap_utils
import concourse.bass as bass
import concourse.bacc as bacc
import concourse.mybir as mybir
import concourse.tile as tile
from concourse.bass import MemorySpace
from concourse.bass_utils import run_bass_kernel_spmd
from concourse._compat import exact_div, get_trn_type, round_up_to_multiple

F32 = mybir.dt.float32
BF16 = mybir.dt.bfloat16
FP8 = mybir.dt.float8e4
I16 = mybir.dt.int16
AF = mybir.ActivationFunctionType
ALU = mybir.AluOpType
NPBF = ml_dtypes.bfloat16
NPF8 = ml_dtypes.float8_e4m3

H = 64
F_IN = 5
L = 3
EPS_BN = 1e-5
EPS_MSG = 1e-7
NCORES = 8
TA = 3
TB = 3
BINCAP = 32
CHUNK_BINS = 16          # bins per gather superchunk
MOCK_COLLECTIVES = False  # replace AllGathers with local DMA (TimelineSim)
NARROW_GATHER = True      # fetch 128B payloads from 256B-stride tables
GATHER_SPLIT = 8          # sub-calls per superchunk gather: 768-idx calls
                          # are the largest the runtime-fixed SWDGE
                          # descriptor ring accepts (larger calls deadlock
                          # the firmware's await_space)
DMA_SCRATCH = 16384       # SWDGE descriptor carveout bytes/partition


# ---------------------------------------------------------------- schedule
class Sched:
    pass


def build_schedule(edge_index, batch_idx, G):
    s = Sched()
    src = np.asarray(edge_index[0], np.int64)
    dst = np.asarray(edge_index[1], np.int64)
    batch = np.asarray(batch_idx, np.int64)
    n = batch.shape[0]
    s.G = G
    s.GPC = GPC = G // NCORES

    deg = np.bincount(dst, minlength=n).astype(np.float64) + 1.0
    s.dinv_node = (deg ** -0.5).astype(np.float32)

    a_edge = batch[src] < (G // 2)
    acnt = np.bincount(dst[a_edge], minlength=n)
    bcnt = np.bincount(dst[~a_edge], minlength=n)

    gstart = np.searchsorted(batch, np.arange(G))
    gend = np.searchsorted(batch, np.arange(G), side="right")
    s.cnt = cnt = gend - gstart

    CAP_A, CAP_B = TA * 128, TB * 128
    core_bins = []
    for c in range(NCORES):
        lo, hi = gstart[c * GPC], gend[(c + 1) * GPC - 1]
        bins, cur, ca, cb = [], [-1, -1], 0, 0
        for nd in range(lo, hi):
            if len(cur) >= BINCAP or ca + acnt[nd] > CAP_A or cb + bcnt[nd] > CAP_B:
                bins.append(cur)
                cur, ca, cb = [], 0, 0
            cur.append(nd)
            ca += acnt[nd]
            cb += bcnt[nd]
        bins.append(cur)
        core_bins.append(bins)

    NB = max(len(b) for b in core_bins)
    # blocks of 4 bins past every core's real bins are pure padding:
    # their gather calls, matmuls, drains and node ops are skipped
    s.NBLK_REAL = -(-NB // 4)
    NB = -(-NB // CHUNK_BINS) * CHUNK_BINS
    s.NB = NB
    s.NSLOT = NSLOT = NB * BINCAP
    s.NBLK = NB // 4
    assert 4 * NSLOT <= 32768, NSLOT

    slot2node = np.full((NCORES, NSLOT), -1, np.int64)
    pos_of_node = np.full(n, -1, np.int64)
    for c in range(NCORES):
        for bi, bn in enumerate(core_bins[c]):
            for j, nd in enumerate(bn):
                if nd >= 0:
                    slot2node[c, bi * BINCAP + j] = nd
                    pos_of_node[nd] = c * NSLOT + bi * BINCAP + j
    assert (pos_of_node >= 0).all()
    s.slot2node, s.pos_of_node = slot2node, pos_of_node
    s.SPLIT = 4 * NSLOT

    dst_pos = pos_of_node[dst]
    dst_core = dst_pos // NSLOT
    dst_bin = (dst_pos % NSLOT) // BINCAP
    dst_slot = (dst_pos % NSLOT) % BINCAP
    src_pos = pos_of_node[src]

    NT_A, NT_B = NB * TA, NB * TB
    idxA = np.zeros((NCORES, NT_A * 128), np.int16)
    dstA = np.full((NCORES, NT_A * 128), -1.0, np.float32)
    idxB = np.zeros((NCORES, NT_B * 128), np.int16)
    dstB = np.full((NCORES, NT_B * 128), -1.0, np.float32)

    order = np.lexsort((src_pos, dst_bin, dst_core))
    eo_src, eo_core = src_pos[order], dst_core[order]
    eo_bin, eo_slot, eo_a = dst_bin[order], dst_slot[order], a_edge[order]

    for c in range(NCORES):
        msk_c = eo_core == c
        for idxarr, dstarr, T, off, grp in (
            (idxA, dstA, TA, 0, True),
            (idxB, dstB, TB, s.SPLIT, False),
        ):
            msk = msk_c & (eo_a == grp)
            bins_e, srcs, slots = eo_bin[msk], eo_src[msk] - off, eo_slot[msk]
            bs = np.searchsorted(bins_e, np.arange(NB))
            be = np.searchsorted(bins_e, np.arange(NB), side="right")
            for bi in range(NB):
                k = be[bi] - bs[bi]
                assert k <= T * 128
                base = bi * T * 128
                idxarr[c, base : base + k] = srcs[bs[bi] : be[bi]].astype(np.int16)
                dstarr[c, base : base + k] = slots[bs[bi] : be[bi]].astype(np.float32)

    s.idxA, s.idxB = idxA, idxB
    # host-built one-hot selection matrices, fp8
    # [K*128] dst codes -> [128, K, 32] one-hot (partition = edge in tile)
    iot = np.arange(BINCAP, dtype=np.float32)

    def onehot(dstarr, ntiles):
        codes = dstarr.reshape(ntiles, 128).T            # [128, ntiles]
        return (codes[:, :, None] == iot[None, None, :]).astype(NPF8)

    s.selA = np.stack([onehot(dstA[c], NT_A) for c in range(NCORES)])
    s.selB = np.stack([onehot(dstB[c], NT_B) for c in range(NCORES)])

    valid = slot2node >= 0
    s.valid = valid
    s.dinv_slot = np.where(
        valid, s.dinv_node[np.clip(slot2node, 0, None)], 0.0
    ).astype(np.float32)
    s.mask_slot = valid.astype(np.float32)

    maxcnt = int(cnt.max())
    SG = max(64, -(-maxcnt // 64) * 64)   # %64 so 2-graph pool gathers are %128
    s.SG = SG
    gidx_mean = np.zeros((NCORES, GPC * SG), np.int16)
    gidx_max = np.zeros((NCORES, GPC * SG), np.int16)
    for c in range(NCORES):
        for gl in range(GPC):
            g = c * GPC + gl
            slots = (pos_of_node[np.arange(gstart[g], gend[g])] % NSLOT).astype(
                np.int16
            )
            base = gl * SG
            gidx_mean[c, base : base + len(slots)] = slots
            gidx_max[c, base : base + len(slots)] = slots
            gidx_mean[c, base + len(slots) : base + SG] = 1
            gidx_max[c, base + len(slots) : base + SG] = 0
    s.gidx_mean, s.gidx_max = gidx_mean, gidx_max
    s.inv_cnt = (1.0 / np.maximum(cnt, 1)).astype(np.float32)
    s.maxmask = (cnt > 0).astype(np.float32)
    return s


def fold_weights(w):
    f = {}
    w32 = {k: np.asarray(v, np.float32) if np.asarray(v).dtype != np.int64 else v
           for k, v in w.items()}
    sbn1 = w32["bn1_g"] / np.sqrt(1.0 + EPS_BN)
    f["Wc"] = (w32["conv1_W"] * sbn1[None, :]).astype(np.float32)
    f["btot_conv"] = (w32["conv1_b"] * sbn1 + w32["bn1_b"]).astype(np.float32)
    f["ln_g"], f["ln_b"] = w32["ln_g"], w32["ln_b"]
    f["prelu_a"], f["gen_t"] = w32["prelu_a"], w32["gen_t"]
    # v-table fast path: relu(prelu(u)) == relu(u) elementwise iff slope > 0,
    # so the message v can be computed before the in-place prelu.
    assert (w32["prelu_a"] > 0).all(), "kernel assumes positive prelu slopes"
    f["g_unit"] = [bool((w32["ln_g"][i] == 1.0).all()) for i in range(L)]
    f["b_zero"] = [bool((w32["ln_b"][i] == 0.0).all()) for i in range(L)]
    f["a_scalar"] = [
        float(w32["prelu_a"][i][0])
        if (w32["prelu_a"][i] == w32["prelu_a"][i][0]).all() else None
        for i in range(L)
    ]
    f["W1"], f["b1tot"], f["W2"], f["b2"] = [], [], [], []
    for i in range(L):
        smlp = w32["mlp_bn_g"][i] / np.sqrt(1.0 + EPS_BN)
        f["W1"].append((w32["mlp_W1"][i] * smlp[None, :]).astype(NPBF))
        f["b1tot"].append(
            (w32["mlp_b1"][i] * smlp + w32["mlp_bn_b"][i]).astype(np.float32)
        )
        f["W2"].append(w32["mlp_W2"][i].astype(NPBF))
        f["b2"].append(w32["mlp_b2"][i])
    for k in ("lin1_W", "lin1_b", "lin2_W", "lin2_b", "out_W", "out_b"):
        f[k] = w32[k]
    return f


def _wrap16(arr):
    """[K*16] -> [128, K] gather-idx layout (i at [i%16, i//16], tiled x8)."""
    a = np.asarray(arr, np.int16).reshape(-1, 16).T  # [16, K]
    return np.tile(a, (8, 1)).copy()


def build_inmaps(s, x):
    NSLOT, NBLK = s.NSLOT, s.NBLK
    maps = []
    for c in range(NCORES):
        # dinv folded in on the host: h0n = (x * dinv) @ Wc
        xpad = np.zeros((NSLOT, F_IN), np.float32)
        v = s.valid[c]
        xpad[v] = (np.asarray(x, np.float32)[s.slot2node[c][v]]
                   * s.dinv_slot[c][v][:, None])
        m = {
            "xT": np.ascontiguousarray(xpad.T),
            "idxA": _wrap16(s.idxA[c]),
            "idxB": _wrap16(s.idxB[c]),
            "selA": np.ascontiguousarray(s.selA[c]),
            "selB": np.ascontiguousarray(s.selB[c]),
            "dinv": np.ascontiguousarray(
                s.dinv_slot[c].reshape(NBLK, 128).T
            ),
            "mask": np.ascontiguousarray(
                s.mask_slot[c].reshape(NBLK, 128).T
            ),
            "gidxm": _wrap16(s.gidx_mean[c]),
            "gidxx": _wrap16(s.gidx_max[c]),
            "pminv": np.tile(s.inv_cnt[c * s.GPC : (c + 1) * s.GPC], (128, 1)).astype(np.float32),
            "pmax": np.tile(s.maxmask[c * s.GPC : (c + 1) * s.GPC], (128, 1)).astype(np.float32),
        }
        maps.append(m)
    return maps


def _dma_gather_narrow(gp, out_ap, in_ap, idxs_ap, num_idxs, num_idxs_reg,
                       elem_size, elem_step, queue_num):
    """bass.GpSimd.dma_gather fork: non-transpose DRAM-source gather whose
    fetched payload (elem_size) is narrower than the 256B-granular row stride
    (elem_step). The stock API asserts elem_size_bytes % 256 == 0, but that is
    a transpose-mode firmware restriction; the non-transpose descriptor
    generator emits one elem_size_bytes descriptor per index with the source
    address advancing by stride_bytes_256*256 per index, so a narrow payload
    on a padded-stride table is well-formed."""
    gp._assert_queue_num(queue_num)
    assert idxs_ap.dtype == mybir.dt.int16
    assert in_ap.space == MemorySpace.DRAM
    assert idxs_ap.space == MemorySpace.SBUF and out_ap.space == MemorySpace.SBUF
    assert in_ap.dtype == out_ap.dtype
    assert ap_utils.ap_is_contiguous(out_ap.ap[1:])
    assert ap_utils.ap_is_contiguous(idxs_ap.ap[1:])
    assert in_ap.ap[-1][1] == out_ap.ap[-1][1] == elem_size
    assert out_ap.ap[0][1] * out_ap.ap[1][1] == round_up_to_multiple(num_idxs, 128)
    assert in_ap.ap[0][0] == elem_step
    stride_bytes_256 = exact_div(elem_step * mybir.dt.size(in_ap.dtype), 256)
    assert stride_bytes_256 < 256
    return gp.add_instruction(
        mybir.InstDMAGatherAnt(
            name=gp.bass.get_next_instruction_name(),
            ins=[
                *gp.lower_ap_dma(in_ap, for_custom_bir_dma=True),
                gp.lower_ap(idxs_ap),
                gp.lower_val_access(gp.to_reg(num_idxs_reg)),
            ],
            outs=[gp.lower_ap(out_ap)],
            transpose=False,
            num_idxs=num_idxs,
            elem_size=elem_size,
            stride_bytes_256=stride_bytes_256,
            gen_mode=0,
            single_packet=True,
            queue_num=queue_num,
            sbuf_tokens_per_rank=0,
            sbuf_free_dim_per_rank=0,
            sbuf_free_dim_pad_per_rank=0,
            sbuf_byte_offset=0,
        )
    )


# ---------------------------------------------------------------- bass build
def build_nc(s, f):
    NB, NSLOT, NBLK, SG, GPC = s.NB, s.NSLOT, s.NBLK, s.SG, s.GPC
    NBLKR = s.NBLK_REAL          # real (non-padding) blocks of 4 bins
    NSC = NB // CHUNK_BINS
    NT_CH_A = CHUNK_BINS * TA            # tiles per A-chunk (48)
    NT_CH_B = CHUNK_BINS * TB
    NIDX_A = NT_CH_A * 128
    NIDX_B = NT_CH_B * 128
    NTA, NTB = NB * TA, NB * TB

    nc = bacc.Bacc(get_trn_type() or "TRN2", num_devices=NCORES,
                   num_swdge_queues=2, dynamic_dma_scratch_size=DMA_SCRATCH)

    # ---- I/O ----
    xT_d = nc.dram_tensor("xT", [F_IN, NSLOT], F32, kind="ExternalInput")
    idxA_d = nc.dram_tensor("idxA", [128, NTA * 8], I16, kind="ExternalInput")
    idxB_d = nc.dram_tensor("idxB", [128, NTB * 8], I16, kind="ExternalInput")
    selA_d = nc.dram_tensor("selA", [128, NTA, 32], FP8, kind="ExternalInput")
    selB_d = nc.dram_tensor("selB", [128, NTB, 32], FP8, kind="ExternalInput")
    dinv_d = nc.dram_tensor("dinv", [128, NBLK], F32, kind="ExternalInput")
    mask_d = nc.dram_tensor("mask", [128, NBLK], F32, kind="ExternalInput")
    gidxm_d = nc.dram_tensor("gidxm", [128, GPC * SG // 16], I16, kind="ExternalInput")
    gidxx_d = nc.dram_tensor("gidxx", [128, GPC * SG // 16], I16, kind="ExternalInput")
    pminv_d = nc.dram_tensor("pminv", [128, GPC], F32, kind="ExternalInput")
    pmax_d = nc.dram_tensor("pmax", [128, GPC], F32, kind="ExternalInput")
    out_d = nc.dram_tensor("out", [s.G, 1], F32, kind="ExternalOutput")

    # ---- shared consts ----
    it = nc.inline_tensor
    Wc_d = it(f["Wc"], "Wc")                                     # [5,64]
    btotb_d = it(np.tile(f["btot_conv"], (128, 1)), "btotb")     # [128,64]
    W1_d = [it(f["W1"][i], f"W1_{i}") for i in range(L)]         # [64,128] bf16
    W2_d = [it(f["W2"][i], f"W2_{i}") for i in range(L)]         # [128,64] bf16
    b1_d = [it(f["b1tot"][i][:, None], f"b1_{i}") for i in range(L)]   # [128,1]
    b2b_d = [it(np.tile(f["b2"][i], (128, 1)), f"b2b_{i}") for i in range(L)]
    gbb_d = [it(np.tile(f["ln_g"][i], (128, 1)), f"gbb_{i}") for i in range(L)]
    bbb_d = [it(np.tile(f["ln_b"][i], (128, 1)), f"bbb_{i}") for i in range(L)]
    abb_d = [it(np.tile(f["prelu_a"][i], (128, 1)), f"abb_{i}") for i in range(L)]
    l1W_d = [it(np.ascontiguousarray(f["lin1_W"][k * 128 : (k + 1) * 128]), f"l1W_{k}") for k in range(4)]
    l1b_d = it(f["lin1_b"][:, None], "l1b")                      # [128,1]
    l2W_d = it(f["lin2_W"], "l2W")                               # [128,64]
    l2b_d = it(f["lin2_b"][:, None], "l2b")                      # [64,1]
    oW_d = it(f["out_W"], "oW")                                  # [64,1]
    ident_d = it(np.eye(128, dtype=np.float32), "ident")

    # ---- internal DRAM ----
    # table rows at 256B stride (gather row-stride granularity): [h0n|-]
    # during conv, [e|ve] during GEN. Double-buffered per layer so layer
    # i+1's AllGather overlaps the tail of layer i's gather stream.
    ag_in = [nc.dram_tensor(f"ag_in{j}", [NSLOT, 128], BF16) for j in range(2)]
    ag_out = [nc.dram_tensor(f"ag_out{j}", [NCORES * NSLOT, 128], BF16,
                             addr_space="Shared") for j in range(2)]
    pool_in = nc.dram_tensor("pool_in", [4, 128, GPC], F32)
    pool_out = nc.dram_tensor("pool_out", [NCORES, 4, 128, GPC], F32, addr_space="Shared")

    RG = [list(range(NCORES))]

    def allgather(cin, cout):
        if MOCK_COLLECTIVES:
            nc.sync.dma_start(out=cout[0 : cin.shape[0]], in_=cin[:])
        else:
            nc.gpsimd.collective_compute(
                "AllGather", ALU.bypass, replica_groups=RG,
                ins=[cin[:]], outs=[cout[:]],
            )

    from contextlib import ExitStack

    with tile.TileContext(nc) as tc:
        with tc.tile_pool(name="persist", bufs=1) as pp:
            # message-passing-lifetime pool: closed before pooling so the
            # pool/head phase can reuse the space
            eph_ctx = ExitStack()
            eh = eph_ctx.enter_context(tc.tile_pool(name="eph", bufs=1))
            idxA_sb = eh.tile([128, NTA * 8], I16)
            nc.sync.dma_start(out=idxA_sb[:], in_=idxA_d[:, :])
            idxB_sb = eh.tile([128, NTB * 8], I16)
            nc.sync.dma_start(out=idxB_sb[:], in_=idxB_d[:, :])
            # sel tiles are declared here but loaded after the conv node
            # phase: they are first read by the conv edge matmuls, so the
            # ~56KB load must not delay the h0/x DMAs and first gathers
            selA = eh.tile([128, NTA, 32], FP8)
            selB = eh.tile([128, NTB, 32], FP8)
            dinv = pp.tile([128, NBLK], F32)
            nc.sync.dma_start(out=dinv[:], in_=dinv_d[:, :])
            mask = pp.tile([128, NBLK], F32)
            nc.sync.dma_start(out=mask[:], in_=mask_d[:, :])

            # consts
            _ldn = [0]

            def ld(dram, shape, dtype=F32):
                _ldn[0] += 1
                nm = f"c{_ldn[0]}_{dram.name}"
                t = pp.tile(shape, dtype, name=nm, tag=nm)
                nc.sync.dma_start(out=t[:], in_=dram[tuple(slice(None) for _ in shape)])
                return t

            Wc = ld(Wc_d, [F_IN, H])
            btotb = ld(btotb_d, [128, H])
            W1 = [ld(W1_d[i], [H, 2 * H], BF16) for i in range(L)]
            W2 = [ld(W2_d[i], [2 * H, H], BF16) for i in range(L)]
            b1 = [ld(b1_d[i], [128, 1]) for i in range(L)]
            b2b = [ld(b2b_d[i], [128, H]) for i in range(L)]
            gbb = [ld(gbb_d[i], [128, H]) for i in range(L)]
            bbb = [ld(bbb_d[i], [128, H]) for i in range(L)]
            abb = [ld(abb_d[i], [128, H]) for i in range(L)]
            l1W = [ld(l1W_d[k], [128, 128]) for k in range(4)]
            l1b = ld(l1b_d, [128, 1])
            l2W = ld(l2W_d, [128, H])
            l2b = ld(l2b_d, [H, 1])
            oW = ld(oW_d, [H, 1])
            ident = ld(ident_d, [128, 128])
            epsb = pp.tile([128, 1], F32)
            nc.vector.memset(epsb[:], EPS_BN)
            e30b = pp.tile([128, 1], F32)
            nc.vector.memset(e30b[:], 1e-30)

            # persistent state
            ledger = pp.tile([128, NBLK, (L + 1) * H], BF16)
            usc = eh.tile([128, NBLK, H], F32)       # h0n during conv, u in GEN
            ab = eh.tile([128, NBLK, 2 * H], BF16)   # table rows: [h0n|-]/[e|ve]

            assert NIDX_A == NIDX_B
            # sub-call tile counts per superchunk: 7-tile (896-idx) calls put
            # 57 descriptors on each 64-entry per-DMA SWDGE ring (49 for the
            # 6-tile tail call); 8-tile calls (65) deadlock the firmware
            CALL_TILES = [7, 7, 7, 7, 7, 7, 6]
            assert sum(CALL_TILES) == NT_CH_A
            nidx_regs = {t: nc.gpsimd.to_reg(t * 128) for t in set(CALL_TILES)}

            def node_chunk(i, sc, ep):
                """LN -> v-table (+ per-chunk ag_in store) -> in-place prelu
                for GEN layer i over superchunk sc's blocks (clamped to the
                real, non-padding blocks). Emitted inside the previous edge
                phase so it overlaps the gather stream."""
                B0 = 4 * sc
                nb = min(4, NBLKR - B0)
                mv = ep.tile([128, 4, 2], F32, tag="nmv", bufs=2)
                for k in range(nb):
                    blk = B0 + k
                    st = ep.tile([128, 6], F32, tag="nst", bufs=3)
                    nc.vector.bn_stats(out=st[:], in_=ledger[:, blk, i * H : (i + 1) * H])
                    nc.vector.bn_aggr(out=mv[:, k, :], in_=st[:])
                rstd = ep.tile([128, 4], F32, tag="nrstd", bufs=2)
                nc.scalar.activation(
                    out=rstd[:, 0:nb], in_=mv[:, 0:nb, 1], func=AF.Sqrt,
                    bias=epsb[:], scale=1.0,
                )
                nc.vector.reciprocal(out=rstd[:, 0:nb], in_=rstd[:, 0:nb])
                nmr = ep.tile([128, 4], F32, tag="nnmr", bufs=2)
                nc.vector.tensor_tensor(
                    out=nmr[:, 0:nb], in0=mv[:, 0:nb, 0], in1=rstd[:, 0:nb],
                    op=ALU.mult,
                )
                nc.vector.tensor_scalar(
                    out=nmr[:, 0:nb], in0=nmr[:, 0:nb], scalar1=-1.0,
                    scalar2=None, op0=ALU.mult,
                )
                for k in range(nb):
                    blk = B0 + k
                    nc.vector.tensor_scalar(
                        out=usc[:, blk, :],
                        in0=ledger[:, blk, i * H : (i + 1) * H],
                        scalar1=rstd[:, k : k + 1],
                        scalar2=nmr[:, k : k + 1],
                        op0=ALU.mult, op1=ALU.add,
                    )
                u = usc[:, B0 : B0 + nb, :]
                if not f["g_unit"][i]:
                    gbig = gbb[i][:].unsqueeze(1).broadcast_to([128, nb, H])
                    nc.vector.tensor_tensor(out=u, in0=u, in1=gbig, op=ALU.mult)
                if not f["b_zero"][i]:
                    bbig = bbb[i][:].unsqueeze(1).broadcast_to([128, nb, H])
                    nc.vector.tensor_tensor(out=u, in0=u, in1=bbig, op=ALU.add)
                # v before the in-place prelu (valid for slopes > 0), then the
                # table rows [e, v*e] in bf16
                vt = ep.tile([128, 4, H], F32, tag="nr", bufs=2)
                nc.vector.tensor_scalar(
                    out=vt[:, 0:nb, :], in0=u, scalar1=0.0, scalar2=EPS_MSG,
                    op0=ALU.max, op1=ALU.add,
                )
                et = ep.tile([128, 4, H], F32, tag="net", bufs=2)
                nc.scalar.activation(
                    out=et[:, 0:nb, :], in_=vt[:, 0:nb, :], func=AF.Exp,
                    scale=float(f["gen_t"][i]),
                )
                nc.vector.tensor_copy(
                    out=ab[:, B0 : B0 + nb, 0:H], in_=et[:, 0:nb, :]
                )
                nc.vector.tensor_tensor(
                    out=ab[:, B0 : B0 + nb, H : 2 * H], in0=vt[:, 0:nb, :],
                    in1=et[:, 0:nb, :], op=ALU.mult,
                )
                nc.sync.dma_start(
                    out=ag_in[(i + 1) % 2].ap()[
                        B0 * 128 : (B0 + nb) * 128, :
                    ].rearrange("(b p) c -> p b c", p=128),
                    in_=ab[:, B0 : B0 + nb, :],
                )
                # prelu in place for the root add; vt = relu(u) + 1e-7 stands
                # in for relu(u) (shift far below tolerance)
                if f["a_scalar"][i] is not None:
                    nc.vector.tensor_scalar(
                        out=u, in0=u, scalar1=0.0, scalar2=f["a_scalar"][i],
                        op0=ALU.min, op1=ALU.mult,
                    )
                else:
                    abig = abb[i][:].unsqueeze(1).broadcast_to([128, nb, H])
                    nc.vector.tensor_tensor(
                        out=u, in0=u, in1=vt[:, 0:nb, :], op=ALU.subtract
                    )
                    nc.vector.tensor_tensor(out=u, in0=u, in1=abig, op=ALU.mult)
                nc.vector.tensor_tensor(out=u, in0=u, in1=vt[:, 0:nb, :], op=ALU.add)

            def edge_phase(tag, nch, drain_fn, tbl, post_chunk=None):
                """Shared edge machinery over the bf16 table rows.

                Conv (nch=H): PSUM[slot, 0:H] = sum_e h0n[src] (narrow 128B
                fetch). GEN (nch=2H): PSUM[slot, :] = sum_e [e|ve][src].
                drain_fn(blk, psum, ep, mpp)."""
                fetch = nch
                with (
                    tc.tile_pool(name=f"ep_{tag}", bufs=1) as ep,
                    tc.tile_pool(name=f"epp_{tag}", bufs=3, space="PSUM") as epp,
                    tc.tile_pool(name=f"mpp_{tag}", bufs=1, space="PSUM") as mpp,
                ):
                    for sc in range(NSC):
                        ia = idxA_sb[:, sc * (NIDX_A // 16) : (sc + 1) * (NIDX_A // 16)]
                        ib = idxB_sb[:, sc * (NIDX_B // 16) : (sc + 1) * (NIDX_B // 16)]
                        ga = ep.tile([128, NT_CH_A, fetch], BF16, tag="ga", bufs=2)
                        gb = ep.tile([128, NT_CH_B, fetch], BF16, tag="gb", bufs=2)
                        st = 0
                        for tpc in CALL_TILES:
                            if sc * NT_CH_A + st >= NBLKR * 4 * TA:
                                break
                            nn = tpc * 128
                            _dma_gather_narrow(
                                nc.gpsimd, ga[:, st : st + tpc, :],
                                tbl[0 : s.SPLIT, 0:fetch],
                                ia[:, st * 8 : st * 8 + nn // 16],
                                nn, nidx_regs[tpc], fetch, 128, queue_num=0,
                            )
                            _dma_gather_narrow(
                                nc.gpsimd, gb[:, st : st + tpc, :],
                                tbl[s.SPLIT : 2 * s.SPLIT, 0:fetch],
                                ib[:, st * 8 : st * 8 + nn // 16],
                                nn, nidx_regs[tpc], fetch, 128, queue_num=1,
                            )
                            st += tpc
                        for bl in range(4):
                            blk = sc * 4 + bl
                            if blk >= NBLKR:
                                continue
                            ps = epp.tile([128, nch], F32, tag="eps", space="PSUM")
                            for j in range(4):
                                lbin = bl * 4 + j       # bin within superchunk
                                for t in range(TA):
                                    gt = lbin * TA + t
                                    nc.tensor.matmul(
                                        out=ps[32 * j : 32 * j + 32, :],
                                        lhsT=selA[:, (sc * CHUNK_BINS + lbin) * TA + t, :],
                                        rhs=ga[:, gt, :],
                                        start=(t == 0),
                                        stop=False,
                                        tile_position=(0, 32 * j),
                                    )
                                for t in range(TB):
                                    gt = lbin * TB + t
                                    nc.tensor.matmul(
                                        out=ps[32 * j : 32 * j + 32, :],
                                        lhsT=selB[:, (sc * CHUNK_BINS + lbin) * TB + t, :],
                                        rhs=gb[:, gt, :],
                                        start=False,
                                        stop=(t == TB - 1),
                                        tile_position=(0, 32 * j),
                                    )
                            drain_fn(blk, ps, ep, mpp)
                        if post_chunk is not None:
                            post_chunk(sc, ep)

            # ================= conv =================
            with (
                tc.tile_pool(name="cvp", bufs=2, space="PSUM") as cvp,
                tc.tile_pool(name="cvs", bufs=1) as cvs,
            ):
                # one DMA for all of x; the cvs pool closes before the edge
                # pools open, so the tile doesn't stack with gather buffers
                xt_all = cvs.tile([F_IN, NSLOT], F32, tag="xt_all")
                nc.sync.dma_start(out=xt_all[:], in_=xT_d[:, :])
                for q in range(-(-NBLKR // 4)):
                    h0ps = cvp.tile([128, 4, H], F32, space="PSUM")
                    for k in range(4):
                        blk = q * 4 + k
                        nc.tensor.matmul(
                            out=h0ps[:, k, :],
                            lhsT=xt_all[:, blk * 128 : (blk + 1) * 128],
                            rhs=Wc[:],
                            start=True, stop=True,
                        )
                    nc.vector.tensor_copy(
                        out=ab[:, q * 4 : q * 4 + 4, 0:H], in_=h0ps[:]
                    )
            nc.sync.dma_start(
                out=ag_in[0].ap()[:, 0:H].rearrange("(b p) c -> p b c", p=128),
                in_=ab[:, :, 0:H],
            )
            allgather(ag_in[0], ag_out[0])
            nc.sync.dma_start(out=selA[:], in_=selA_d[:, :, :])
            nc.sync.dma_start(out=selB[:], in_=selB_d[:, :, :])

            def conv_drain(blk, ps, ep, mpp):
                t1 = ep.tile([128, H], F32, tag="cd", bufs=3)
                nc.vector.tensor_add(t1[:], ps[:], ab[:, blk, 0:H])
                nc.vector.tensor_scalar(
                    out=t1[:], in0=t1[:],
                    scalar1=dinv[:, blk : blk + 1], scalar2=None, op0=ALU.mult,
                )
                nc.vector.tensor_add(t1[:], t1[:], btotb[:])
                nc.vector.tensor_scalar(
                    out=ledger[:, blk, 0:H], in0=t1[:],
                    scalar1=0.0, scalar2=mask[:, blk : blk + 1],
                    op0=ALU.max, op1=ALU.mult,
                )

            edge_phase("cv", H, conv_drain, tbl=ag_out[0],
                       post_chunk=lambda sc, ep: node_chunk(0, sc, ep))
            allgather(ag_in[1], ag_out[1])

            # ================= GEN layers =================
            # node phase for layer i is interleaved into the previous edge
            # phase (post_chunk); only the table AllGather sits between.
            for i in range(L):
                def gen_drain(blk, ps, ep, mpp, i=i):
                    sden = ep.tile([128, H], F32, tag="sden", bufs=3)
                    nc.vector.tensor_scalar(
                        out=sden[:], in0=ps[:, 0:H], scalar1=1e-30, scalar2=None,
                        op0=ALU.add,
                    )
                    nc.vector.reciprocal(out=sden[:], in_=sden[:])
                    agg = ep.tile([128, H], F32, tag="agg", bufs=3)
                    nc.vector.tensor_tensor(
                        out=agg[:], in0=ps[:, H : 2 * H], in1=sden[:], op=ALU.mult
                    )
                    nc.vector.tensor_add(agg[:], agg[:], usc[:, blk, :])
                    tps = mpp.tile([H, 128], F32, tag="tps", space="PSUM")
                    nc.tensor.transpose(out=tps[:], in_=agg[:], identity=ident[:])
                    aggT = ep.tile([H, 128], BF16, tag="aggT", bufs=3)
                    nc.vector.tensor_copy(out=aggT[:], in_=tps[:])
                    z1ps = mpp.tile([128, 128], F32, tag="z1", space="PSUM")
                    nc.tensor.matmul(
                        out=z1ps[:], lhsT=W1[i][:], rhs=aggT[:], start=True, stop=True
                    )
                    z1r = ep.tile([128, 128], BF16, tag="z1r", bufs=3)
                    nc.scalar.activation(
                        out=z1r[:], in_=z1ps[:], func=AF.Relu, bias=b1[i][:], scale=1.0
                    )
                    z2ps = mpp.tile([128, H], F32, tag="z2", space="PSUM")
                    nc.tensor.matmul(
                        out=z2ps[:], lhsT=z1r[:], rhs=W2[i][:], start=True, stop=True
                    )
                    t2 = ep.tile([128, H], F32, tag="t2", bufs=3)
                    nc.vector.tensor_add(t2[:], z2ps[:], b2b[i][:])
                    nc.vector.tensor_add(t2[:], t2[:], ledger[:, blk, i * H : (i + 1) * H])
                    nc.vector.tensor_scalar(
                        out=ledger[:, blk, (i + 1) * H : (i + 2) * H], in0=t2[:],
                        scalar1=mask[:, blk : blk + 1], scalar2=None, op0=ALU.mult,
                    )

                edge_phase(
                    f"g{i}", 2 * H, gen_drain, tbl=ag_out[(i + 1) % 2],
                    post_chunk=(
                        (lambda sc, ep, j=i + 1: node_chunk(j, sc, ep))
                        if i < L - 1 else None
                    ),
                )
                if i < L - 1:
                    allgather(ag_in[(i + 2) % 2], ag_out[(i + 2) % 2])

            # ================= pooling + head =================
            eph_ctx.close()
            CH = (L + 1) * H
            with (
                tc.tile_pool(name="pool", bufs=1) as qp,
                tc.tile_pool(name="poolps", bufs=2, space="PSUM") as qpp,
            ):
                gnidx_reg = nc.gpsimd.to_reg(2 * SG)
                nc.vector.memset(ledger[0:1, 0, 0:CH], -3.0e38)
                lbf = ledger
                pooled = qp.tile([128, 4, GPC], F32)
                PGS = 2 * SG                      # idxs per sub-call (<=768)
                gis, pscs, grids = [], [], []
                for which, gidx_d, pscale_d in (
                    (0, gidxm_d, pminv_d),
                    (1, gidxx_d, pmax_d),
                ):
                    gi = qp.tile([128, GPC * SG // 16], I16, tag=f"gi{which}")
                    nc.sync.dma_start(out=gi[:], in_=gidx_d[:, :])
                    psc = qp.tile([128, GPC], F32, tag=f"psc{which}")
                    nc.sync.dma_start(out=psc[:], in_=pscale_d[:, :])
                    grid = qp.tile([128, GPC // 2, 2, PGS], BF16, tag=f"grid{which}")
                    gis.append(gi)
                    pscs.append(psc)
                    grids.append(grid)
                # issue mean/max gathers interleaved on the two queues and
                # reduce each 2-graph slab as soon as its gather lands, so the
                # reduces trail the gather stream instead of following it
                for k in range(GPC // 2):
                    for which in (0, 1):
                        nc.gpsimd.dma_gather(
                            grids[which][:, k, :, :],
                            lbf[:].rearrange("p b c -> p (b c)"),
                            gis[which][:, k * (PGS // 16) : (k + 1) * (PGS // 16)],
                            PGS, gnidx_reg, CH,
                            transpose=True,
                            sbuf_tokens_per_rank=128,
                            sbuf_free_dim_per_rank=CH * 2,
                            queue_num=which,
                        )
                    for which in (0, 1):
                        red_op = (nc.vector.reduce_sum if which == 0
                                  else nc.vector.reduce_max)
                        for half in range(2):
                            red = qp.tile([128, 2], F32, tag="red", bufs=4)
                            red_op(
                                out=red[:].rearrange("p (k m) -> p k m", k=1),
                                in_=grids[which][:, k, half, :].rearrange(
                                    "p (m t) -> p () m t", t=SG
                                ),
                                axis=mybir.AxisListType.X,
                            )
                            nc.vector.tensor_tensor(
                                out=pooled[:, which * 2 + half, 2 * k : 2 * k + 2],
                                in0=red[:],
                                in1=pscs[which][:, 2 * k : 2 * k + 2],
                                op=ALU.mult,
                            )
                nc.sync.dma_start(
                    out=pool_in.ap().rearrange("k p g -> p k g"), in_=pooled[:]
                )
                if MOCK_COLLECTIVES:
                    nc.sync.dma_start(
                        out=pool_out[0, :, :, :], in_=pool_in[:, :, :]
                    )
                else:
                    nc.gpsimd.collective_compute(
                        "AllGather", ALU.bypass, replica_groups=RG,
                        ins=[pool_in[:, :, :]], outs=[pool_out[:, :, :, :]],
                    )
                # head
                hps = qpp.tile([128, s.G], F32, tag="hps", space="PSUM")
                pk = []
                for k in range(4):
                    t = qp.tile([128, NCORES, GPC], F32, tag=f"pk{k}")
                    nc.sync.dma_start(
                        out=t[:], in_=pool_out[:, k, :, :].rearrange("r p g -> p r g")
                    )
                    pk.append(t)
                for k in range(4):
                    nc.tensor.matmul(
                        out=hps[:], lhsT=l1W[k][:],
                        rhs=pk[k][:].rearrange("p r g -> p (r g)"),
                        start=(k == 0), stop=(k == 3),
                    )
                hz1 = qp.tile([128, s.G], F32)
                nc.scalar.activation(
                    out=hz1[:], in_=hps[:], func=AF.Relu, bias=l1b[:], scale=1.0
                )
                h2ps = qpp.tile([H, s.G], F32, tag="h2ps", space="PSUM")
                nc.tensor.matmul(out=h2ps[:], lhsT=l2W[:], rhs=hz1[:], start=True, stop=True)
                hz2 = qp.tile([H, s.G], F32)
                nc.scalar.activation(
                    out=hz2[:], in_=h2ps[:], func=AF.Relu, bias=l2b[:], scale=1.0
                )
                ops = qpp.tile([1, s.G], F32, tag="ops", space="PSUM")
                nc.tensor.matmul(out=ops[:], lhsT=oW[:], rhs=hz2[:], start=True, stop=True)
                osb = qp.tile([1, s.G], F32)
                nc.vector.tensor_scalar(
                    out=osb[:], in0=ops[:], scalar1=float(f["out_b"][0]),
                    scalar2=None, op0=ALU.add,
                )
                nc.sync.dma_start(out=out_d.ap().rearrange("g one -> one g"), in_=osb[:])

    nc.compile()
    return nc


# ---------------------------------------------------------------- entry
def kernel(**inputs) -> np.ndarray:
    x = np.asarray(inputs["x"], np.float32)
    ei = np.asarray(inputs["edge_index"], np.int64)
    bi = np.asarray(inputs["batch_idx"], np.int64)
    G = 256
    s = build_schedule(ei, bi, G)
    f = fold_weights(inputs)
    maps = build_inmaps(s, x)
    nc = build_nc(s, f)
    res = run_bass_kernel_spmd(nc, maps, core_ids=list(range(NCORES)))
    return np.asarray(res.results[0]["out"], np.float32)


# revision 75
# speedup vs baseline: 1.1379x; 1.0030x over previous
"""Trainium2 Bass kernel for nn_GCN_5403068858882 (GCN + 3x GENConv + pool head).

Self-contained: schedule builder + bass program builder + SPMD runner.

Design:
- 8 cores, core c owns graphs [32c, 32c+32) (contiguous nodes, batch sorted).
- Nodes packed into 32-slot bins (cap TA*128 "A" edges / TB*128 "B" edges,
  A = src graph < G/2 so dma_gather int16 indices fit a half table).
- Selection matrices (one-hot of edge->dst-slot) are precomputed on the host
  in fp8 and loaded once as resident SBUF inputs; the PE matmul pairs them
  with the bf16 gathered rows (mixed-dtype matmul), so no on-device
  is_equal selection builds exist.
- Per GEN layer: node-space LN -> v=relu(u)+eps (before the in-place prelu,
  valid for positive slopes) -> bf16 table rows [e, v*e] -> AllGather
  (double-buffered across layers) -> per 128-edge tile: dma_gather rows
  (256B payloads; 896-idx calls put 57 descriptors on each 64-entry
  per-DMA SWDGE ring, the firmware limit) + PE matmul with the resident selection matrix
  accumulating numerator/denominator in PSUM -> agg=w/s+u -> MLP (bn
  folded, bf16) -> residual ledger (bf16).
- The node phase of layer i+1 is emitted per-superchunk inside layer i's
  edge phase (post_chunk) so it overlaps the gather stream.
- GCN conv: same machinery with narrow 128B fetches of bf16 h0*(deg^-1/2)
  rows (dinv folded into x on the host); self loop via own-row add.
- Pooling: bf16 SBUF-source dma_gather (transpose) straight from the bf16
  ledger into a per-graph padded channel-major grid, with per-slab reduces
  trailing the gather stream -> tiny AllGather -> MLP head.
"""

import numpy as np
import ml_dtypes

import concourse.ap_utils as ap_utils
import concourse.bass as bass
import concourse.bacc as bacc
import concourse.mybir as mybir
import concourse.tile as tile
from concourse.bass import MemorySpace
from concourse.bass_utils import run_bass_kernel_spmd
from concourse._compat import exact_div, get_trn_type, round_up_to_multiple

F32 = mybir.dt.float32
BF16 = mybir.dt.bfloat16
FP8 = mybir.dt.float8e4
I16 = mybir.dt.int16
AF = mybir.ActivationFunctionType
ALU = mybir.AluOpType
NPBF = ml_dtypes.bfloat16
NPF8 = ml_dtypes.float8_e4m3

H = 64
F_IN = 5
L = 3
EPS_BN = 1e-5
EPS_MSG = 1e-7
NCORES = 8
TA = 3
TB = 3
BINCAP = 32
CHUNK_BINS = 16          # bins per gather superchunk
MOCK_COLLECTIVES = False  # replace AllGathers with local DMA (TimelineSim)
NARROW_GATHER = True      # fetch 128B payloads from 256B-stride tables
GATHER_SPLIT = 8          # sub-calls per superchunk gather: 768-idx calls
                          # are the largest the runtime-fixed SWDGE
                          # descriptor ring accepts (larger calls deadlock
                          # the firmware's await_space)
DMA_SCRATCH = 16384       # SWDGE descriptor carveout bytes/partition


# ---------------------------------------------------------------- schedule
class Sched:
    pass


def build_schedule(edge_index, batch_idx, G):
    s = Sched()
    src = np.asarray(edge_index[0], np.int64)
    dst = np.asarray(edge_index[1], np.int64)
    batch = np.asarray(batch_idx, np.int64)
    n = batch.shape[0]
    s.G = G
    s.GPC = GPC = G // NCORES

    deg = np.bincount(dst, minlength=n).astype(np.float64) + 1.0
    s.dinv_node = (deg ** -0.5).astype(np.float32)

    a_edge = batch[src] < (G // 2)
    acnt = np.bincount(dst[a_edge], minlength=n)
    bcnt = np.bincount(dst[~a_edge], minlength=n)

    gstart = np.searchsorted(batch, np.arange(G))
    gend = np.searchsorted(batch, np.arange(G), side="right")
    s.cnt = cnt = gend - gstart

    CAP_A, CAP_B = TA * 128, TB * 128
    core_bins = []
    for c in range(NCORES):
        lo, hi = gstart[c * GPC], gend[(c + 1) * GPC - 1]
        bins, cur, ca, cb = [], [-1, -1], 0, 0
        for nd in range(lo, hi):
            if len(cur) >= BINCAP or ca + acnt[nd] > CAP_A or cb + bcnt[nd] > CAP_B:
                bins.append(cur)
                cur, ca, cb = [], 0, 0
            cur.append(nd)
            ca += acnt[nd]
            cb += bcnt[nd]
        bins.append(cur)
        core_bins.append(bins)

    NB = max(len(b) for b in core_bins)
    # blocks of 4 bins past every core's real bins are pure padding:
    # their gather calls, matmuls, drains and node ops are skipped
    s.NBLK_REAL = -(-NB // 4)
    NB = -(-NB // CHUNK_BINS) * CHUNK_BINS
    s.NB = NB
    s.NSLOT = NSLOT = NB * BINCAP
    s.NBLK = NB // 4
    assert 4 * NSLOT <= 32768, NSLOT

    slot2node = np.full((NCORES, NSLOT), -1, np.int64)
    pos_of_node = np.full(n, -1, np.int64)
    for c in range(NCORES):
        for bi, bn in enumerate(core_bins[c]):
            for j, nd in enumerate(bn):
                if nd >= 0:
                    slot2node[c, bi * BINCAP + j] = nd
                    pos_of_node[nd] = c * NSLOT + bi * BINCAP + j
    assert (pos_of_node >= 0).all()
    s.slot2node, s.pos_of_node = slot2node, pos_of_node
    s.SPLIT = 4 * NSLOT

    dst_pos = pos_of_node[dst]
    dst_core = dst_pos // NSLOT
    dst_bin = (dst_pos % NSLOT) // BINCAP
    dst_slot = (dst_pos % NSLOT) % BINCAP
    src_pos = pos_of_node[src]

    NT_A, NT_B = NB * TA, NB * TB
    idxA = np.zeros((NCORES, NT_A * 128), np.int16)
    dstA = np.full((NCORES, NT_A * 128), -1.0, np.float32)
    idxB = np.zeros((NCORES, NT_B * 128), np.int16)
    dstB = np.full((NCORES, NT_B * 128), -1.0, np.float32)

    order = np.lexsort((src_pos, dst_bin, dst_core))
    eo_src, eo_core = src_pos[order], dst_core[order]
    eo_bin, eo_slot, eo_a = dst_bin[order], dst_slot[order], a_edge[order]

    for c in range(NCORES):
        msk_c = eo_core == c
        for idxarr, dstarr, T, off, grp in (
            (idxA, dstA, TA, 0, True),
            (idxB, dstB, TB, s.SPLIT, False),
        ):
            msk = msk_c & (eo_a == grp)
            bins_e, srcs, slots = eo_bin[msk], eo_src[msk] - off, eo_slot[msk]
            bs = np.searchsorted(bins_e, np.arange(NB))
            be = np.searchsorted(bins_e, np.arange(NB), side="right")
            for bi in range(NB):
                k = be[bi] - bs[bi]
                assert k <= T * 128
                base = bi * T * 128
                idxarr[c, base : base + k] = srcs[bs[bi] : be[bi]].astype(np.int16)
                dstarr[c, base : base + k] = slots[bs[bi] : be[bi]].astype(np.float32)

    s.idxA, s.idxB = idxA, idxB
    # host-built one-hot selection matrices, fp8
    # [K*128] dst codes -> [128, K, 32] one-hot (partition = edge in tile)
    iot = np.arange(BINCAP, dtype=np.float32)

    def onehot(dstarr, ntiles):
        codes = dstarr.reshape(ntiles, 128).T            # [128, ntiles]
        return (codes[:, :, None] == iot[None, None, :]).astype(NPF8)

    s.selA = np.stack([onehot(dstA[c], NT_A) for c in range(NCORES)])
    s.selB = np.stack([onehot(dstB[c], NT_B) for c in range(NCORES)])

    valid = slot2node >= 0
    s.valid = valid
    s.dinv_slot = np.where(
        valid, s.dinv_node[np.clip(slot2node, 0, None)], 0.0
    ).astype(np.float32)
    s.mask_slot = valid.astype(np.float32)

    maxcnt = int(cnt.max())
    SG = max(64, -(-maxcnt // 64) * 64)   # %64 so 2-graph pool gathers are %128
    s.SG = SG
    gidx_mean = np.zeros((NCORES, GPC * SG), np.int16)
    gidx_max = np.zeros((NCORES, GPC * SG), np.int16)
    for c in range(NCORES):
        for gl in range(GPC):
            g = c * GPC + gl
            slots = (pos_of_node[np.arange(gstart[g], gend[g])] % NSLOT).astype(
                np.int16
            )
            base = gl * SG
            gidx_mean[c, base : base + len(slots)] = slots
            gidx_max[c, base : base + len(slots)] = slots
            gidx_mean[c, base + len(slots) : base + SG] = 1
            gidx_max[c, base + len(slots) : base + SG] = 0
    s.gidx_mean, s.gidx_max = gidx_mean, gidx_max
    s.inv_cnt = (1.0 / np.maximum(cnt, 1)).astype(np.float32)
    s.maxmask = (cnt > 0).astype(np.float32)
    return s


def fold_weights(w):
    f = {}
    w32 = {k: np.asarray(v, np.float32) if np.asarray(v).dtype != np.int64 else v
           for k, v in w.items()}
    sbn1 = w32["bn1_g"] / np.sqrt(1.0 + EPS_BN)
    f["Wc"] = (w32["conv1_W"] * sbn1[None, :]).astype(np.float32)
    f["btot_conv"] = (w32["conv1_b"] * sbn1 + w32["bn1_b"]).astype(np.float32)
    f["ln_g"], f["ln_b"] = w32["ln_g"], w32["ln_b"]
    f["prelu_a"], f["gen_t"] = w32["prelu_a"], w32["gen_t"]
    # v-table fast path: relu(prelu(u)) == relu(u) elementwise iff slope > 0,
    # so the message v can be computed before the in-place prelu.
    assert (w32["prelu_a"] > 0).all(), "kernel assumes positive prelu slopes"
    f["g_unit"] = [bool((w32["ln_g"][i] == 1.0).all()) for i in range(L)]
    f["b_zero"] = [bool((w32["ln_b"][i] == 0.0).all()) for i in range(L)]
    f["a_scalar"] = [
        float(w32["prelu_a"][i][0])
        if (w32["prelu_a"][i] == w32["prelu_a"][i][0]).all() else None
        for i in range(L)
    ]
    f["W1"], f["b1tot"], f["W2"], f["b2"] = [], [], [], []
    for i in range(L):
        smlp = w32["mlp_bn_g"][i] / np.sqrt(1.0 + EPS_BN)
        f["W1"].append((w32["mlp_W1"][i] * smlp[None, :]).astype(NPBF))
        f["b1tot"].append(
            (w32["mlp_b1"][i] * smlp + w32["mlp_bn_b"][i]).astype(np.float32)
        )
        f["W2"].append(w32["mlp_W2"][i].astype(NPBF))
        f["b2"].append(w32["mlp_b2"][i])
    for k in ("lin1_W", "lin2_W", "out_W"):
        f[k] = w32[k].astype(NPBF)
    for k in ("lin1_b", "lin2_b", "out_b"):
        f[k] = w32[k]
    return f


def _wrap16(arr):
    """[K*16] -> [128, K] gather-idx layout (i at [i%16, i//16], tiled x8)."""
    a = np.asarray(arr, np.int16).reshape(-1, 16).T  # [16, K]
    return np.tile(a, (8, 1)).copy()


def build_inmaps(s, x):
    NSLOT, NBLK = s.NSLOT, s.NBLK
    maps = []
    for c in range(NCORES):
        # dinv folded in on the host: h0n = (x * dinv) @ Wc
        xpad = np.zeros((NSLOT, F_IN), np.float32)
        v = s.valid[c]
        xpad[v] = (np.asarray(x, np.float32)[s.slot2node[c][v]]
                   * s.dinv_slot[c][v][:, None])
        m = {
            "xT": np.ascontiguousarray(xpad.T),
            "idxA": _wrap16(s.idxA[c]),
            "idxB": _wrap16(s.idxB[c]),
            "selA": np.ascontiguousarray(s.selA[c]),
            "selB": np.ascontiguousarray(s.selB[c]),
            "dinv": np.ascontiguousarray(
                s.dinv_slot[c].reshape(NBLK, 128).T
            ),
            "mask": np.ascontiguousarray(
                s.mask_slot[c].reshape(NBLK, 128).T
            ),
            "gidxm": _wrap16(s.gidx_mean[c]),
            "gidxx": _wrap16(s.gidx_max[c]),
            "pminv": np.tile(s.inv_cnt[c * s.GPC : (c + 1) * s.GPC], (128, 1)).astype(np.float32),
            "pmax": np.tile(s.maxmask[c * s.GPC : (c + 1) * s.GPC], (128, 1)).astype(np.float32),
        }
        maps.append(m)
    return maps


def _dma_gather_narrow(gp, out_ap, in_ap, idxs_ap, num_idxs, num_idxs_reg,
                       elem_size, elem_step, queue_num):
    """bass.GpSimd.dma_gather fork: non-transpose DRAM-source gather whose
    fetched payload (elem_size) is narrower than the 256B-granular row stride
    (elem_step). The stock API asserts elem_size_bytes % 256 == 0, but that is
    a transpose-mode firmware restriction; the non-transpose descriptor
    generator emits one elem_size_bytes descriptor per index with the source
    address advancing by stride_bytes_256*256 per index, so a narrow payload
    on a padded-stride table is well-formed."""
    gp._assert_queue_num(queue_num)
    assert idxs_ap.dtype == mybir.dt.int16
    assert in_ap.space == MemorySpace.DRAM
    assert idxs_ap.space == MemorySpace.SBUF and out_ap.space == MemorySpace.SBUF
    assert in_ap.dtype == out_ap.dtype
    assert ap_utils.ap_is_contiguous(out_ap.ap[1:])
    assert ap_utils.ap_is_contiguous(idxs_ap.ap[1:])
    assert in_ap.ap[-1][1] == out_ap.ap[-1][1] == elem_size
    assert out_ap.ap[0][1] * out_ap.ap[1][1] == round_up_to_multiple(num_idxs, 128)
    assert in_ap.ap[0][0] == elem_step
    stride_bytes_256 = exact_div(elem_step * mybir.dt.size(in_ap.dtype), 256)
    assert stride_bytes_256 < 256
    return gp.add_instruction(
        mybir.InstDMAGatherAnt(
            name=gp.bass.get_next_instruction_name(),
            ins=[
                *gp.lower_ap_dma(in_ap, for_custom_bir_dma=True),
                gp.lower_ap(idxs_ap),
                gp.lower_val_access(gp.to_reg(num_idxs_reg)),
            ],
            outs=[gp.lower_ap(out_ap)],
            transpose=False,
            num_idxs=num_idxs,
            elem_size=elem_size,
            stride_bytes_256=stride_bytes_256,
            gen_mode=0,
            single_packet=True,
            queue_num=queue_num,
            sbuf_tokens_per_rank=0,
            sbuf_free_dim_per_rank=0,
            sbuf_free_dim_pad_per_rank=0,
            sbuf_byte_offset=0,
        )
    )


# ---------------------------------------------------------------- bass build
def build_nc(s, f):
    NB, NSLOT, NBLK, SG, GPC = s.NB, s.NSLOT, s.NBLK, s.SG, s.GPC
    NBLKR = s.NBLK_REAL          # real (non-padding) blocks of 4 bins
    NSC = NB // CHUNK_BINS
    NT_CH_A = CHUNK_BINS * TA            # tiles per A-chunk (48)
    NT_CH_B = CHUNK_BINS * TB
    NIDX_A = NT_CH_A * 128
    NIDX_B = NT_CH_B * 128
    NTA, NTB = NB * TA, NB * TB

    nc = bacc.Bacc(get_trn_type() or "TRN2", num_devices=NCORES,
                   num_swdge_queues=2, dynamic_dma_scratch_size=DMA_SCRATCH)

    # ---- I/O ----
    xT_d = nc.dram_tensor("xT", [F_IN, NSLOT], F32, kind="ExternalInput")
    idxA_d = nc.dram_tensor("idxA", [128, NTA * 8], I16, kind="ExternalInput")
    idxB_d = nc.dram_tensor("idxB", [128, NTB * 8], I16, kind="ExternalInput")
    selA_d = nc.dram_tensor("selA", [128, NTA, 32], FP8, kind="ExternalInput")
    selB_d = nc.dram_tensor("selB", [128, NTB, 32], FP8, kind="ExternalInput")
    dinv_d = nc.dram_tensor("dinv", [128, NBLK], F32, kind="ExternalInput")
    mask_d = nc.dram_tensor("mask", [128, NBLK], F32, kind="ExternalInput")
    gidxm_d = nc.dram_tensor("gidxm", [128, GPC * SG // 16], I16, kind="ExternalInput")
    gidxx_d = nc.dram_tensor("gidxx", [128, GPC * SG // 16], I16, kind="ExternalInput")
    pminv_d = nc.dram_tensor("pminv", [128, GPC], F32, kind="ExternalInput")
    pmax_d = nc.dram_tensor("pmax", [128, GPC], F32, kind="ExternalInput")
    out_d = nc.dram_tensor("out", [s.G, 1], F32, kind="ExternalOutput")

    # ---- shared consts ----
    it = nc.inline_tensor
    Wc_d = it(f["Wc"], "Wc")                                     # [5,64]
    btotb_d = it(np.tile(f["btot_conv"], (128, 1)), "btotb")     # [128,64]
    W1_d = [it(f["W1"][i], f"W1_{i}") for i in range(L)]         # [64,128] bf16
    W2_d = [it(f["W2"][i], f"W2_{i}") for i in range(L)]         # [128,64] bf16
    b1_d = [it(f["b1tot"][i][:, None], f"b1_{i}") for i in range(L)]   # [128,1]
    b2b_d = [it(np.tile(f["b2"][i], (128, 1)), f"b2b_{i}") for i in range(L)]
    gbb_d = [it(np.tile(f["ln_g"][i], (128, 1)), f"gbb_{i}") for i in range(L)]
    bbb_d = [it(np.tile(f["ln_b"][i], (128, 1)), f"bbb_{i}") for i in range(L)]
    abb_d = [it(np.tile(f["prelu_a"][i], (128, 1)), f"abb_{i}") for i in range(L)]
    l1W_d = [it(np.ascontiguousarray(f["lin1_W"][k * 128 : (k + 1) * 128]), f"l1W_{k}") for k in range(4)]
    l1b_d = it(f["lin1_b"][:, None], "l1b")                      # [128,1]
    l2W_d = it(f["lin2_W"], "l2W")                               # [128,64]
    l2b_d = it(f["lin2_b"][:, None], "l2b")                      # [64,1]
    oW_d = it(f["out_W"], "oW")                                  # [64,1]
    ident_d = it(np.eye(128, dtype=np.float32), "ident")

    # ---- internal DRAM ----
    # table rows at 256B stride (gather row-stride granularity): [h0n|-]
    # during conv, [e|ve] during GEN. Double-buffered per layer so layer
    # i+1's AllGather overlaps the tail of layer i's gather stream.
    ag_in = [nc.dram_tensor(f"ag_in{j}", [NSLOT, 128], BF16) for j in range(2)]
    ag_out = [nc.dram_tensor(f"ag_out{j}", [NCORES * NSLOT, 128], BF16,
                             addr_space="Shared") for j in range(2)]
    pool_in = nc.dram_tensor("pool_in", [4, 128, GPC], BF16)
    pool_out = nc.dram_tensor("pool_out", [NCORES, 4, 128, GPC], BF16, addr_space="Shared")

    RG = [list(range(NCORES))]

    def allgather(cin, cout):
        if MOCK_COLLECTIVES:
            nc.sync.dma_start(out=cout[0 : cin.shape[0]], in_=cin[:])
        else:
            nc.gpsimd.collective_compute(
                "AllGather", ALU.bypass, replica_groups=RG,
                ins=[cin[:]], outs=[cout[:]],
            )

    from contextlib import ExitStack

    with tile.TileContext(nc) as tc:
        with tc.tile_pool(name="persist", bufs=1) as pp:
            # message-passing-lifetime pool: closed before pooling so the
            # pool/head phase can reuse the space
            eph_ctx = ExitStack()
            eh = eph_ctx.enter_context(tc.tile_pool(name="eph", bufs=1))
            idxA_sb = eh.tile([128, NTA * 8], I16)
            nc.sync.dma_start(out=idxA_sb[:], in_=idxA_d[:, :])
            idxB_sb = eh.tile([128, NTB * 8], I16)
            nc.sync.dma_start(out=idxB_sb[:], in_=idxB_d[:, :])
            # sel tiles are declared here but loaded after the conv node
            # phase: they are first read by the conv edge matmuls, so the
            # ~56KB load must not delay the h0/x DMAs and first gathers
            selA = eh.tile([128, NTA, 32], FP8)
            selB = eh.tile([128, NTB, 32], FP8)
            dinv = pp.tile([128, NBLK], F32)
            nc.sync.dma_start(out=dinv[:], in_=dinv_d[:, :])
            mask = pp.tile([128, NBLK], F32)
            nc.sync.dma_start(out=mask[:], in_=mask_d[:, :])

            # consts
            _ldn = [0]

            def ld(dram, shape, dtype=F32):
                _ldn[0] += 1
                nm = f"c{_ldn[0]}_{dram.name}"
                t = pp.tile(shape, dtype, name=nm, tag=nm)
                nc.sync.dma_start(out=t[:], in_=dram[tuple(slice(None) for _ in shape)])
                return t

            Wc = ld(Wc_d, [F_IN, H])
            btotb = ld(btotb_d, [128, H])
            W1 = [ld(W1_d[i], [H, 2 * H], BF16) for i in range(L)]
            W2 = [ld(W2_d[i], [2 * H, H], BF16) for i in range(L)]
            b1 = [ld(b1_d[i], [128, 1]) for i in range(L)]
            b2b = [ld(b2b_d[i], [128, H]) for i in range(L)]
            gbb = [ld(gbb_d[i], [128, H]) for i in range(L)]
            bbb = [ld(bbb_d[i], [128, H]) for i in range(L)]
            abb = [ld(abb_d[i], [128, H]) for i in range(L)]
            l1W = [ld(l1W_d[k], [128, 128], BF16) for k in range(4)]
            l1b = ld(l1b_d, [128, 1])
            l2W = ld(l2W_d, [128, H], BF16)
            l2b = ld(l2b_d, [H, 1])
            oW = ld(oW_d, [H, 1], BF16)
            ident = ld(ident_d, [128, 128])
            epsb = pp.tile([128, 1], F32)
            nc.vector.memset(epsb[:], EPS_BN)
            e30b = pp.tile([128, 1], F32)
            nc.vector.memset(e30b[:], 1e-30)

            # persistent state
            ledger = pp.tile([128, NBLK, (L + 1) * H], BF16)
            usc = eh.tile([128, NBLK, H], F32)       # h0n during conv, u in GEN
            ab = eh.tile([128, NBLK, 2 * H], BF16)   # table rows: [h0n|-]/[e|ve]

            assert NIDX_A == NIDX_B
            # sub-call tile counts per superchunk: 7-tile (896-idx) calls put
            # 57 descriptors on each 64-entry per-DMA SWDGE ring (49 for the
            # 6-tile tail call); 8-tile calls (65) deadlock the firmware
            CALL_TILES = [7, 7, 7, 7, 7, 7, 6]
            assert sum(CALL_TILES) == NT_CH_A
            nidx_regs = {t: nc.gpsimd.to_reg(t * 128) for t in set(CALL_TILES)}

            def node_chunk(i, sc, ep):
                """LN -> v-table (+ per-chunk ag_in store) -> in-place prelu
                for GEN layer i over superchunk sc's blocks (clamped to the
                real, non-padding blocks). Emitted inside the previous edge
                phase so it overlaps the gather stream."""
                B0 = 4 * sc
                nb = min(4, NBLKR - B0)
                mv = ep.tile([128, 4, 2], F32, tag="nmv", bufs=2)
                for k in range(nb):
                    blk = B0 + k
                    st = ep.tile([128, 6], F32, tag="nst", bufs=3)
                    nc.vector.bn_stats(out=st[:], in_=ledger[:, blk, i * H : (i + 1) * H])
                    nc.vector.bn_aggr(out=mv[:, k, :], in_=st[:])
                rstd = ep.tile([128, 4], F32, tag="nrstd", bufs=2)
                nc.scalar.activation(
                    out=rstd[:, 0:nb], in_=mv[:, 0:nb, 1], func=AF.Sqrt,
                    bias=epsb[:], scale=1.0,
                )
                nc.vector.reciprocal(out=rstd[:, 0:nb], in_=rstd[:, 0:nb])
                nmr = ep.tile([128, 4], F32, tag="nnmr", bufs=2)
                nc.vector.tensor_tensor(
                    out=nmr[:, 0:nb], in0=mv[:, 0:nb, 0], in1=rstd[:, 0:nb],
                    op=ALU.mult,
                )
                nc.vector.tensor_scalar(
                    out=nmr[:, 0:nb], in0=nmr[:, 0:nb], scalar1=-1.0,
                    scalar2=None, op0=ALU.mult,
                )
                for k in range(nb):
                    blk = B0 + k
                    nc.vector.tensor_scalar(
                        out=usc[:, blk, :],
                        in0=ledger[:, blk, i * H : (i + 1) * H],
                        scalar1=rstd[:, k : k + 1],
                        scalar2=nmr[:, k : k + 1],
                        op0=ALU.mult, op1=ALU.add,
                    )
                u = usc[:, B0 : B0 + nb, :]
                if not f["g_unit"][i]:
                    gbig = gbb[i][:].unsqueeze(1).broadcast_to([128, nb, H])
                    nc.vector.tensor_tensor(out=u, in0=u, in1=gbig, op=ALU.mult)
                if not f["b_zero"][i]:
                    bbig = bbb[i][:].unsqueeze(1).broadcast_to([128, nb, H])
                    nc.vector.tensor_tensor(out=u, in0=u, in1=bbig, op=ALU.add)
                # v before the in-place prelu (valid for slopes > 0), then the
                # table rows [e, v*e] in bf16
                vt = ep.tile([128, 4, H], F32, tag="nr", bufs=2)
                nc.vector.tensor_scalar(
                    out=vt[:, 0:nb, :], in0=u, scalar1=0.0, scalar2=EPS_MSG,
                    op0=ALU.max, op1=ALU.add,
                )
                et = ep.tile([128, 4, H], F32, tag="net", bufs=2)
                nc.scalar.activation(
                    out=et[:, 0:nb, :], in_=vt[:, 0:nb, :], func=AF.Exp,
                    scale=float(f["gen_t"][i]),
                )
                nc.vector.tensor_copy(
                    out=ab[:, B0 : B0 + nb, 0:H], in_=et[:, 0:nb, :]
                )
                nc.vector.tensor_tensor(
                    out=ab[:, B0 : B0 + nb, H : 2 * H], in0=vt[:, 0:nb, :],
                    in1=et[:, 0:nb, :], op=ALU.mult,
                )
                nc.sync.dma_start(
                    out=ag_in[(i + 1) % 2].ap()[
                        B0 * 128 : (B0 + nb) * 128, :
                    ].rearrange("(b p) c -> p b c", p=128),
                    in_=ab[:, B0 : B0 + nb, :],
                )
                # prelu in place for the root add; vt = relu(u) + 1e-7 stands
                # in for relu(u) (shift far below tolerance)
                if f["a_scalar"][i] is not None:
                    nc.vector.tensor_scalar(
                        out=u, in0=u, scalar1=0.0, scalar2=f["a_scalar"][i],
                        op0=ALU.min, op1=ALU.mult,
                    )
                else:
                    abig = abb[i][:].unsqueeze(1).broadcast_to([128, nb, H])
                    nc.vector.tensor_tensor(
                        out=u, in0=u, in1=vt[:, 0:nb, :], op=ALU.subtract
                    )
                    nc.vector.tensor_tensor(out=u, in0=u, in1=abig, op=ALU.mult)
                nc.vector.tensor_tensor(out=u, in0=u, in1=vt[:, 0:nb, :], op=ALU.add)

            def edge_phase(tag, nch, drain_fn, tbl, post_chunk=None):
                """Shared edge machinery over the bf16 table rows.

                Conv (nch=H): PSUM[slot, 0:H] = sum_e h0n[src] (narrow 128B
                fetch). GEN (nch=2H): PSUM[slot, :] = sum_e [e|ve][src].
                drain_fn(blk, psum, ep, mpp)."""
                fetch = nch
                with (
                    tc.tile_pool(name=f"ep_{tag}", bufs=1) as ep,
                    tc.tile_pool(name=f"epp_{tag}", bufs=3, space="PSUM") as epp,
                    tc.tile_pool(name=f"mpp_{tag}", bufs=1, space="PSUM") as mpp,
                ):
                    for sc in range(NSC):
                        ia = idxA_sb[:, sc * (NIDX_A // 16) : (sc + 1) * (NIDX_A // 16)]
                        ib = idxB_sb[:, sc * (NIDX_B // 16) : (sc + 1) * (NIDX_B // 16)]
                        ga = ep.tile([128, NT_CH_A, fetch], BF16, tag="ga", bufs=2)
                        gb = ep.tile([128, NT_CH_B, fetch], BF16, tag="gb", bufs=2)
                        st = 0
                        for tpc in CALL_TILES:
                            if sc * NT_CH_A + st >= NBLKR * 4 * TA:
                                break
                            nn = tpc * 128
                            _dma_gather_narrow(
                                nc.gpsimd, ga[:, st : st + tpc, :],
                                tbl[0 : s.SPLIT, 0:fetch],
                                ia[:, st * 8 : st * 8 + nn // 16],
                                nn, nidx_regs[tpc], fetch, 128, queue_num=0,
                            )
                            _dma_gather_narrow(
                                nc.gpsimd, gb[:, st : st + tpc, :],
                                tbl[s.SPLIT : 2 * s.SPLIT, 0:fetch],
                                ib[:, st * 8 : st * 8 + nn // 16],
                                nn, nidx_regs[tpc], fetch, 128, queue_num=1,
                            )
                            st += tpc
                        for bl in range(4):
                            blk = sc * 4 + bl
                            if blk >= NBLKR:
                                continue
                            ps = epp.tile([128, nch], F32, tag="eps", space="PSUM")
                            for j in range(4):
                                lbin = bl * 4 + j       # bin within superchunk
                                for t in range(TA):
                                    gt = lbin * TA + t
                                    nc.tensor.matmul(
                                        out=ps[32 * j : 32 * j + 32, :],
                                        lhsT=selA[:, (sc * CHUNK_BINS + lbin) * TA + t, :],
                                        rhs=ga[:, gt, :],
                                        start=(t == 0),
                                        stop=False,
                                        tile_position=(0, 32 * j),
                                    )
                                for t in range(TB):
                                    gt = lbin * TB + t
                                    nc.tensor.matmul(
                                        out=ps[32 * j : 32 * j + 32, :],
                                        lhsT=selB[:, (sc * CHUNK_BINS + lbin) * TB + t, :],
                                        rhs=gb[:, gt, :],
                                        start=False,
                                        stop=(t == TB - 1),
                                        tile_position=(0, 32 * j),
                                    )
                            drain_fn(blk, ps, ep, mpp)
                        if post_chunk is not None:
                            post_chunk(sc, ep)

            # ================= conv =================
            with (
                tc.tile_pool(name="cvp", bufs=2, space="PSUM") as cvp,
                tc.tile_pool(name="cvs", bufs=1) as cvs,
            ):
                # one DMA for all of x; the cvs pool closes before the edge
                # pools open, so the tile doesn't stack with gather buffers
                xt_all = cvs.tile([F_IN, NSLOT], F32, tag="xt_all")
                nc.sync.dma_start(out=xt_all[:], in_=xT_d[:, :])
                for q in range(-(-NBLKR // 4)):
                    h0ps = cvp.tile([128, 4, H], F32, space="PSUM")
                    for k in range(4):
                        blk = q * 4 + k
                        nc.tensor.matmul(
                            out=h0ps[:, k, :],
                            lhsT=xt_all[:, blk * 128 : (blk + 1) * 128],
                            rhs=Wc[:],
                            start=True, stop=True,
                        )
                    nc.vector.tensor_copy(
                        out=ab[:, q * 4 : q * 4 + 4, 0:H], in_=h0ps[:]
                    )
            nc.sync.dma_start(
                out=ag_in[0].ap()[:, 0:H].rearrange("(b p) c -> p b c", p=128),
                in_=ab[:, :, 0:H],
            )
            allgather(ag_in[0], ag_out[0])
            nc.sync.dma_start(out=selA[:], in_=selA_d[:, :, :])
            nc.sync.dma_start(out=selB[:], in_=selB_d[:, :, :])

            def conv_drain(blk, ps, ep, mpp):
                t1 = ep.tile([128, H], F32, tag="cd", bufs=3)
                nc.vector.tensor_add(t1[:], ps[:], ab[:, blk, 0:H])
                nc.vector.tensor_scalar(
                    out=t1[:], in0=t1[:],
                    scalar1=dinv[:, blk : blk + 1], scalar2=None, op0=ALU.mult,
                )
                nc.vector.tensor_add(t1[:], t1[:], btotb[:])
                nc.vector.tensor_scalar(
                    out=ledger[:, blk, 0:H], in0=t1[:],
                    scalar1=0.0, scalar2=mask[:, blk : blk + 1],
                    op0=ALU.max, op1=ALU.mult,
                )

            edge_phase("cv", H, conv_drain, tbl=ag_out[0],
                       post_chunk=lambda sc, ep: node_chunk(0, sc, ep))
            allgather(ag_in[1], ag_out[1])

            # ================= GEN layers =================
            # node phase for layer i is interleaved into the previous edge
            # phase (post_chunk); only the table AllGather sits between.
            for i in range(L):
                def gen_drain(blk, ps, ep, mpp, i=i):
                    sden = ep.tile([128, H], F32, tag="sden", bufs=3)
                    nc.vector.tensor_scalar(
                        out=sden[:], in0=ps[:, 0:H], scalar1=1e-30, scalar2=None,
                        op0=ALU.add,
                    )
                    nc.vector.reciprocal(out=sden[:], in_=sden[:])
                    agg = ep.tile([128, H], F32, tag="agg", bufs=3)
                    nc.vector.tensor_tensor(
                        out=agg[:], in0=ps[:, H : 2 * H], in1=sden[:], op=ALU.mult
                    )
                    nc.vector.tensor_add(agg[:], agg[:], usc[:, blk, :])
                    tps = mpp.tile([H, 128], F32, tag="tps", space="PSUM")
                    nc.tensor.transpose(out=tps[:], in_=agg[:], identity=ident[:])
                    aggT = ep.tile([H, 128], BF16, tag="aggT", bufs=3)
                    nc.vector.tensor_copy(out=aggT[:], in_=tps[:])
                    z1ps = mpp.tile([128, 128], F32, tag="z1", space="PSUM")
                    nc.tensor.matmul(
                        out=z1ps[:], lhsT=W1[i][:], rhs=aggT[:], start=True, stop=True
                    )
                    z1r = ep.tile([128, 128], BF16, tag="z1r", bufs=3)
                    nc.scalar.activation(
                        out=z1r[:], in_=z1ps[:], func=AF.Relu, bias=b1[i][:], scale=1.0
                    )
                    z2ps = mpp.tile([128, H], F32, tag="z2", space="PSUM")
                    nc.tensor.matmul(
                        out=z2ps[:], lhsT=z1r[:], rhs=W2[i][:], start=True, stop=True
                    )
                    t2 = ep.tile([128, H], F32, tag="t2", bufs=3)
                    nc.vector.tensor_add(t2[:], z2ps[:], b2b[i][:])
                    nc.vector.tensor_add(t2[:], t2[:], ledger[:, blk, i * H : (i + 1) * H])
                    nc.vector.tensor_scalar(
                        out=ledger[:, blk, (i + 1) * H : (i + 2) * H], in0=t2[:],
                        scalar1=mask[:, blk : blk + 1], scalar2=None, op0=ALU.mult,
                    )

                edge_phase(
                    f"g{i}", 2 * H, gen_drain, tbl=ag_out[(i + 1) % 2],
                    post_chunk=(
                        (lambda sc, ep, j=i + 1: node_chunk(j, sc, ep))
                        if i < L - 1 else None
                    ),
                )
                if i < L - 1:
                    allgather(ag_in[(i + 2) % 2], ag_out[(i + 2) % 2])

            # ================= pooling + head =================
            eph_ctx.close()
            CH = (L + 1) * H
            with (
                tc.tile_pool(name="pool", bufs=1) as qp,
                tc.tile_pool(name="poolps", bufs=2, space="PSUM") as qpp,
            ):
                gnidx_reg = nc.gpsimd.to_reg(2 * SG)
                nc.vector.memset(ledger[0:1, 0, 0:CH], -3.0e38)
                lbf = ledger
                pooled = qp.tile([128, 4, GPC], BF16)
                PGS = 2 * SG                      # idxs per sub-call (<=768)
                gis, pscs, grids = [], [], []
                for which, gidx_d, pscale_d in (
                    (0, gidxm_d, pminv_d),
                    (1, gidxx_d, pmax_d),
                ):
                    gi = qp.tile([128, GPC * SG // 16], I16, tag=f"gi{which}")
                    nc.sync.dma_start(out=gi[:], in_=gidx_d[:, :])
                    psc = qp.tile([128, GPC], F32, tag=f"psc{which}")
                    nc.sync.dma_start(out=psc[:], in_=pscale_d[:, :])
                    grid = qp.tile([128, GPC // 2, 2, PGS], BF16, tag=f"grid{which}")
                    gis.append(gi)
                    pscs.append(psc)
                    grids.append(grid)
                # issue mean/max gathers interleaved on the two queues and
                # reduce each 2-graph slab as soon as its gather lands, so the
                # reduces trail the gather stream instead of following it
                for k in range(GPC // 2):
                    for which in (0, 1):
                        nc.gpsimd.dma_gather(
                            grids[which][:, k, :, :],
                            lbf[:].rearrange("p b c -> p (b c)"),
                            gis[which][:, k * (PGS // 16) : (k + 1) * (PGS // 16)],
                            PGS, gnidx_reg, CH,
                            transpose=True,
                            sbuf_tokens_per_rank=128,
                            sbuf_free_dim_per_rank=CH * 2,
                            queue_num=which,
                        )
                    for which in (0, 1):
                        red_op = (nc.vector.reduce_sum if which == 0
                                  else nc.vector.reduce_max)
                        for half in range(2):
                            red = qp.tile([128, 2], F32, tag="red", bufs=4)
                            red_op(
                                out=red[:].rearrange("p (k m) -> p k m", k=1),
                                in_=grids[which][:, k, half, :].rearrange(
                                    "p (m t) -> p () m t", t=SG
                                ),
                                axis=mybir.AxisListType.X,
                            )
                            nc.vector.tensor_tensor(
                                out=pooled[:, which * 2 + half, 2 * k : 2 * k + 2],
                                in0=red[:],
                                in1=pscs[which][:, 2 * k : 2 * k + 2],
                                op=ALU.mult,
                            )
                nc.sync.dma_start(
                    out=pool_in.ap().rearrange("k p g -> p k g"), in_=pooled[:]
                )
                if MOCK_COLLECTIVES:
                    nc.sync.dma_start(
                        out=pool_out[0, :, :, :], in_=pool_in[:, :, :]
                    )
                else:
                    nc.gpsimd.collective_compute(
                        "AllGather", ALU.bypass, replica_groups=RG,
                        ins=[pool_in[:, :, :]], outs=[pool_out[:, :, :, :]],
                    )
                # head
                hps = qpp.tile([128, s.G], F32, tag="hps", space="PSUM")
                pk = []
                for k in range(4):
                    t = qp.tile([128, NCORES, GPC], BF16, tag=f"pk{k}")
                    nc.sync.dma_start(
                        out=t[:], in_=pool_out[:, k, :, :].rearrange("r p g -> p r g")
                    )
                    pk.append(t)
                for k in range(4):
                    nc.tensor.matmul(
                        out=hps[:], lhsT=l1W[k][:],
                        rhs=pk[k][:].rearrange("p r g -> p (r g)"),
                        start=(k == 0), stop=(k == 3),
                    )
                hz1 = qp.tile([128, s.G], BF16)
                nc.scalar.activation(
                    out=hz1[:], in_=hps[:], func=AF.Relu, bias=l1b[:], scale=1.0
                )
                h2ps = qpp.tile([H, s.G], F32, tag="h2ps", space="PSUM")
                nc.tensor.matmul(out=h2ps[:], lhsT=l2W[:], rhs=hz1[:], start=True, stop=True)
                hz2 = qp.tile([H, s.G], BF16)
                nc.scalar.activation(
                    out=hz2[:], in_=h2ps[:], func=AF.Relu, bias=l2b[:], scale=1.0
                )
                ops = qpp.tile([1, s.G], F32, tag="ops", space="PSUM")
                nc.tensor.matmul(out=ops[:], lhsT=oW[:], rhs=hz2[:], start=True, stop=True)
                osb = qp.tile([1, s.G], F32)
                nc.vector.tensor_scalar(
                    out=osb[:], in0=ops[:], scalar1=float(f["out_b"][0]),
                    scalar2=None, op0=ALU.add,
                )
                nc.sync.dma_start(out=out_d.ap().rearrange("g one -> one g"), in_=osb[:])

    nc.compile()
    return nc


# ---------------------------------------------------------------- entry
def kernel(**inputs) -> np.ndarray:
    x = np.asarray(inputs["x"], np.float32)
    ei = np.asarray(inputs["edge_index"], np.int64)
    bi = np.asarray(inputs["batch_idx"], np.int64)
    G = 256
    s = build_schedule(ei, bi, G)
    f = fold_weights(inputs)
    maps = build_inmaps(s, x)
    nc = build_nc(s, f)
    res = run_bass_kernel_spmd(nc, maps, core_ids=list(range(NCORES)))
    return np.asarray(res.results[0]["out"], np.float32)
